# revision 12
# baseline (speedup 1.0000x reference)
"""DeepseekMoE Trainium2 kernel — routed 3-launch pipeline on 8 NeuronCores.

All FFN matmuls run as fp8(e4m3) DoubleRow tensor ops (0.5 cycles/row, 256-wide
contraction per instruction) with per-operand hi/lo residual splits choosing a
precision tier per (token, expert-slot):
  R  (320 cyc/tok): x hi/lo, W hi/lo, a1/a2 single fp8  — high combine weight
  B1 (256 cyc/tok): like R but x single fp8             — mid weight
  P8 (128 cyc/tok): everything single fp8               — low weight (w2<=0.35)
The shared expert runs scheme F (R plus an a2 hi/lo split).  Weight hi/lo
splits are host-side parameter preprocessing; the only data-dependent splits
(x, shared a2) are computed on device.

Launch A (data-parallel): fp32 gate (top-2 via sigmoid identity
  w1 = sigmoid(s1 - s2)), device x hi/lo split, shared-expert layer 1.
Launch B (expert-parallel, one expert per core): 3-layer FFN over
  host-gathered tokens in three tier chunks; outputs weighted fp16.
Launch C (data-parallel): shared layers 2+3 and final combine
  out = shared + cont1 + cont2.
Host code between launches only moves data (gather/scatter/layout); all
per-token arithmetic is on device.
"""
import numpy as np
import ml_dtypes
import concourse.mybir as mybir
import concourse.tile as tile
from concourse import bacc
from concourse.bass_utils import run_bass_kernel_spmd

F32 = mybir.dt.float32
F16 = mybir.dt.float16
F8 = mybir.dt.float8e4
E4 = ml_dtypes.float8_e4m3
DR = mybir.MatmulPerfMode.DoubleRow
AL = mybir.AluOpType
Relu = mybir.ActivationFunctionType.Relu
Sigm = mybir.ActivationFunctionType.Sigmoid

N_TOKENS, D, H, O, E = 8192, 1024, 2048, 1024, 8
N_CORES, TOK = 8, 1024
KD, KH = D // 128, H // 128          # contraction 128-blocks
KDP, KHP = KD // 2, KH // 2          # DoubleRow k-pairs
MH, MO = H // 128, O // 128          # output 128-tiles
GT = TOK // 128                      # gate token tiles per core
EPS = 1e-5
SW, SA = 32.0, 8.0                   # weight / activation fp8 storage scales
BIG = 1e30
THR1, THR2 = 0.6, 0.35               # tier thresholds on combine weight


# ---------------------------------------------------------------- host prep
def _wlayout(V):
    """V [K, M] fp32 -> [MT, 128, KP, 2, 2, 128] e4m3 hi/lo DoubleRow layout.
    k = j*256 + ksub*128 + p ; m = mi*128 + mm ; dim4 = (hi, lo)."""
    Kd, Md = V.shape
    KP, MT = Kd // 256, Md // 128
    s = (V * SW).astype(np.float32)
    hi = s.astype(E4)
    lo = (s - hi.astype(np.float32)).astype(E4)
    out = np.empty((MT, 128, KP, 2, 2, 128), E4)
    out[..., 0, :] = hi.reshape(KP, 2, 128, MT, 128).transpose(3, 2, 0, 1, 4)
    out[..., 1, :] = lo.reshape(KP, 2, 128, MT, 128).transpose(3, 2, 0, 1, 4)
    return np.ascontiguousarray(out)


def _fold_params(inp):
    """Fold eval-mode BN into weights; emit fp8 hi/lo layouts + scaled biases."""
    out = []
    for e in range(E + 1):
        if e < E:
            W1, b1 = inp['W1'][e], inp['b1'][e]
            g1, be1, m1, v1 = inp['g1'][e], inp['be1'][e], inp['m1'][e], inp['v1'][e]
            W2, b2 = inp['W2'][e], inp['b2'][e]
            g2, be2, m2, v2 = inp['g2'][e], inp['be2'][e], inp['m2'][e], inp['v2'][e]
            W3, b3 = inp['W3'][e], inp['b3'][e]
        else:
            W1, b1 = inp['sW1'], inp['sb1']
            g1, be1, m1, v1 = inp['sg1'], inp['sbe1'], inp['sm1'], inp['sv1']
            W2, b2 = inp['sW2'], inp['sb2']
            g2, be2, m2, v2 = inp['sg2'], inp['sbe2'], inp['sm2'], inp['sv2']
            W3, b3 = inp['sW3'], inp['sb3']
        s1 = g1 / np.sqrt(v1 + EPS); t1 = be1 - m1 * s1
        s2 = g2 / np.sqrt(v2 + EPS); t2 = be2 - m2 * s2
        V1 = W1.T.astype(np.float32)
        V2 = (s1[:, None] * W2.T).astype(np.float32)
        V3 = (s2[:, None] * W3.T).astype(np.float32)
        c1 = (SA * b1).astype(np.float32)
        c2 = (SA * (b2 + t1 @ W2.T)).astype(np.float32)
        c3 = (b3 + t2 @ W3.T).astype(np.float32)
        out.append(dict(
            V1=_wlayout(V1), V2=_wlayout(V2), V3=_wlayout(V3),
            c1=np.ascontiguousarray(c1.reshape(MH, 128).T),
            c2=np.ascontiguousarray(c2.reshape(MH, 128).T),
            c3=np.ascontiguousarray(c3.reshape(MO, 128).T)))
    return out


# ------------------------------------------------------------ layer builder
def _mm_group(nc, ps, wt, KP, mode, mov, nsl, emit, mi, tag):
    nn = nsl.stop - nsl.start
    pp = ps.tile([128, 512], F32, tag="ps", name=f"pp{tag}")[:, :nn]
    seq = []
    for j in range(KP):
        hi_st = wt[:, j, :, 0, :]
        lo_st = wt[:, j, :, 1, :]
        if mode == "p8":
            seq.append((hi_st, mov(j, 0, nsl)))
        elif mode == "ws":
            mh = mov(j, 0, nsl)
            seq.append((hi_st, mh))
            seq.append((lo_st, mh))
        else:  # both-split
            mh, ml = mov(j, 0, nsl), mov(j, 1, nsl)
            seq += [(hi_st, mh), (hi_st, ml), (lo_st, mh), (lo_st, ml)]
    for i, (st, mv) in enumerate(seq):
        nc.tensor.matmul(pp, st, mv, start=(i == 0),
                         stop=(i == len(seq) - 1), perf_mode=DR)
    emit(mi, nsl, pp)


def _slices(chunks):
    out = []
    for (off, size, mode) in chunks:
        s = 0
        while s < size:
            nn = min(256, size - s)
            out.append((slice(off + s, off + s + nn), mode))
            s += nn
    return out


def _run_layer(nc, wpool, ps, wdram, MT, KP, chunks, mov, emit, tag):
    """One FFN layer over token chunks (mi-outer, streaming weights).
    chunks: [(off, size, mode)]; mov(j, hl, nsl) -> [128, 2, nn] AP."""
    sls = _slices(chunks)
    for mi in range(MT):
        wt = wpool.tile([128, KP, 2, 2, 128], F8, tag="wt", name=f"wt{tag}")
        nc.sync.dma_start(wt, wdram[mi])
        for (nsl, mode) in sls:
            _mm_group(nc, ps, wt, KP, mode, mov, nsl, emit, mi, tag)


# ------------------------------------------------------------ kernel builders
def _build_A():
    """Gate (fp32) + x hi/lo split + shared-expert layer 1."""
    nc = bacc.Bacc("TRN2", target_bir_lowering=False, debug=False,
                   num_devices=N_CORES)
    xT_d = nc.dram_tensor("xT", [D, TOK], F32, kind="ExternalInput")
    wg_d = nc.dram_tensor("WgT", [D, E], F32, kind="ExternalInput")
    V1_d = nc.dram_tensor("V1s", [MH, 128, KDP, 2, 2, 128], F8, kind="ExternalInput")
    C1_d = nc.dram_tensor("C1s", [128, MH], F32, kind="ExternalInput")
    ws_d = nc.dram_tensor("wsum", [128, GT * E], F32, kind="ExternalOutput")
    xhl_d = nc.dram_tensor("xhl", [128, KD, 2, TOK], F8, kind="ExternalOutput")
    a1s_d = nc.dram_tensor("a1s", [TOK // 256, 128, MH, 256], F8,
                           kind="ExternalOutput")

    NB = TOK // 256
    with tile.TileContext(nc) as tc:
        with tc.tile_pool(name="const", bufs=1) as cpool, \
             tc.tile_pool(name="gate", bufs=1) as gpool, \
             tc.tile_pool(name="ps", bufs=8, space="PSUM") as ps:
            x32 = cpool.tile([128, KD, TOK], F32, name="x32")
            for kb in range(KD):
                for b in range(NB):
                    csl = slice(b * 256, (b + 1) * 256)
                    nc.sync.dma_start(
                        x32[:, kb, csl],
                        xT_d.ap().rearrange("(k p) t -> p k t", p=128)[:, kb, csl])
            wg = cpool.tile([128, KD, E], F32, name="wg")
            nc.sync.dma_start(wg, wg_d.ap().rearrange("(k p) e -> p k e", p=128))
            c1_sb = cpool.tile([128, MH], F32, name="c1_sb")
            nc.sync.dma_start(c1_sb, C1_d.ap())
            # resident layer-1 weights (32 KiB/partition)
            wt1 = cpool.tile([128, MH, KDP, 2, 2, 128], F8, name="wt1")
            for mi in range(MH):
                nc.sync.dma_start(wt1[:, mi], V1_d.ap()[mi])

            # x hi/lo split (column-blocked: Act does hi, DVE does lo)
            xhl = cpool.tile([128, KD, 2, TOK], F8, name="xhl")
            for kb in range(KD):
                for b in range(NB):
                    csl = slice(b * 256, (b + 1) * 256)
                    nc.scalar.activation(xhl[:, kb, 0, csl], x32[:, kb, csl],
                                         mybir.ActivationFunctionType.Copy,
                                         bias=0.0, scale=1.0)
                    nc.vector.tensor_tensor(xhl[:, kb, 1, csl], x32[:, kb, csl],
                                            xhl[:, kb, 0, csl], op=AL.subtract)
            nc.sync.dma_start(xhl_d.ap(), xhl)

            # gate scores -> batched top-2 sigmoid weights
            sg = gpool.tile([128, GT, E], F32, name="sg")
            for ti in range(GT):
                tsl = slice(ti * 128, (ti + 1) * 128)
                pg = ps.tile([128, 512], F32, tag="ps", name="pg")[:, :E]
                for kb in range(KD):
                    nc.tensor.matmul(pg, x32[:, kb, tsl], wg[:, kb],
                                     start=(kb == 0), stop=(kb == KD - 1))
                nc.vector.tensor_copy(sg[:, ti], pg)
            m1 = gpool.tile([128, GT, 1], F32, name="m1")
            nc.vector.tensor_reduce(m1, sg, axis=mybir.AxisListType.X, op=AL.max)
            msk1 = gpool.tile([128, GT, E], F32, name="msk1")
            nc.vector.tensor_tensor(msk1, sg, m1.to_broadcast((128, GT, E)),
                                    op=AL.is_equal)
            pen = gpool.tile([128, GT, E], F32, name="pen")
            nc.vector.tensor_scalar_mul(pen, msk1, -BIG)
            nc.vector.tensor_tensor(pen, sg, pen, op=AL.add)
            m2 = gpool.tile([128, GT, 1], F32, name="m2")
            nc.vector.tensor_reduce(m2, pen, axis=mybir.AxisListType.X, op=AL.max)
            dm = gpool.tile([128, GT, 1], F32, name="dm")
            nc.vector.tensor_tensor(dm, m1, m2, op=AL.subtract)
            w1 = gpool.tile([128, GT, 1], F32, name="w1")
            nc.scalar.activation(w1, dm, Sigm, bias=0.0, scale=1.0)
            msk2 = gpool.tile([128, GT, E], F32, name="msk2")
            nc.vector.tensor_tensor(msk2, pen, m2.to_broadcast((128, GT, E)),
                                    op=AL.is_equal)
            t1 = gpool.tile([128, GT, E], F32, name="t1")
            nc.vector.tensor_tensor(t1, msk1, w1.to_broadcast((128, GT, E)),
                                    op=AL.mult)
            w2 = gpool.tile([128, GT, 1], F32, name="w2")
            nc.vector.tensor_scalar(w2, w1, -1.0, 1.0, op0=AL.mult, op1=AL.add)
            t2 = gpool.tile([128, GT, E], F32, name="t2")
            nc.vector.tensor_tensor(t2, msk2, w2.to_broadcast((128, GT, E)),
                                    op=AL.mult)
            wsm = gpool.tile([128, GT, E], F32, name="wsm")
            nc.vector.tensor_tensor(wsm, t1, t2, op=AL.add)
            nc.sync.dma_start(ws_d.ap(), wsm.rearrange("p a b -> p (a b)"))

            # shared expert layer 1 (both-split), block-outer so output
            # blocks stream out as they complete
            a1s = cpool.tile([128, MH, TOK], F8, name="a1s")

            def emit1(mi, nsl, pp):
                nc.scalar.activation(a1s[:, mi, nsl], pp, Relu,
                                     bias=c1_sb[:, mi:mi + 1], scale=SA / SW)

            mov1 = lambda j, hl, nsl: xhl[:, 2 * j:2 * j + 2, hl, nsl]
            for b in range(NB):
                csl = slice(b * 256, (b + 1) * 256)
                for mi in range(MH):
                    _mm_group(nc, ps, wt1[:, mi], KDP, "bs", mov1, csl,
                              emit1, mi, "1")
                nc.sync.dma_start(a1s_d.ap()[b], a1s[:, :, csl])
    nc.compile()
    return nc


def _build_B(caps):
    """One expert per core over gathered tokens in tier chunks (R, B1, P8)."""
    capR, capB, capP = caps
    CT = capR + capB + capP
    NBX = -(-CT // 256)
    nc = bacc.Bacc("TRN2", target_bir_lowering=False, debug=False,
                   num_devices=N_CORES)
    xg_d = nc.dram_tensor("xg", [NBX, 128, KD, 2, 256], F8, kind="ExternalInput")
    wr_d = nc.dram_tensor("wrow", [CT], F16, kind="ExternalInput")
    V1_d = nc.dram_tensor("V1s", [MH, 128, KDP, 2, 2, 128], F8, kind="ExternalInput")
    V2_d = nc.dram_tensor("V2s", [MH, 128, KHP, 2, 2, 128], F8, kind="ExternalInput")
    V3_d = nc.dram_tensor("V3s", [MO, 128, KHP, 2, 2, 128], F8, kind="ExternalInput")
    C1_d = nc.dram_tensor("C1s", [128, MH], F32, kind="ExternalInput")
    C2_d = nc.dram_tensor("C2s", [128, MH], F32, kind="ExternalInput")
    C3_d = nc.dram_tensor("C3s", [128, MO], F32, kind="ExternalInput")
    outb_d = nc.dram_tensor("outb", [O, CT], F16, kind="ExternalOutput")

    ch1 = [(0, capR, "bs"), (capR, capB, "ws"), (capR + capB, capP, "p8")]
    ch23 = [(0, capR, "ws"), (capR, capB, "ws"), (capR + capB, capP, "p8")]

    with tile.TileContext(nc) as tc:
        with tc.tile_pool(name="const", bufs=1) as cpool, \
             tc.tile_pool(name="acts", bufs=1) as apool, \
             tc.tile_pool(name="wts", bufs=4) as wpool, \
             tc.tile_pool(name="tmp", bufs=4) as tpool, \
             tc.tile_pool(name="ps", bufs=8, space="PSUM") as ps:
            xg = cpool.tile([128, KD, 2, NBX * 256], F8, name="xg")
            for b in range(NBX):
                nc.sync.dma_start(xg[:, :, :, b * 256:(b + 1) * 256], xg_d.ap()[b])
            wbc = cpool.tile([128, CT], F16, name="wbc")
            nc.sync.dma_start(wbc, wr_d.ap()[None, :].to_broadcast((128, CT)))
            c1_sb = cpool.tile([128, MH], F32, name="c1_sb")
            nc.sync.dma_start(c1_sb, C1_d.ap())
            c2_sb = cpool.tile([128, MH], F32, name="c2_sb")
            nc.sync.dma_start(c2_sb, C2_d.ap())
            c3_sb = cpool.tile([128, MO], F32, name="c3_sb")
            nc.sync.dma_start(c3_sb, C3_d.ap())

            a1 = apool.tile([128, KH, CT], F8, name="a1")
            a2 = apool.tile([128, KH, CT], F8, name="a2")

            def emit1(mi, nsl, pp):
                nc.scalar.activation(a1[:, mi, nsl], pp, Relu,
                                     bias=c1_sb[:, mi:mi + 1], scale=SA / SW)

            _run_layer(nc, wpool, ps, V1_d.ap(), MH, KDP, ch1,
                       lambda j, hl, nsl: xg[:, 2 * j:2 * j + 2, hl, nsl],
                       emit1, "1")

            def emit2(mi, nsl, pp):
                nc.scalar.activation(a2[:, mi, nsl], pp, Relu,
                                     bias=c2_sb[:, mi:mi + 1], scale=1.0 / SW)

            _run_layer(nc, wpool, ps, V2_d.ap(), MH, KHP, ch23,
                       lambda j, hl, nsl: a1[:, 2 * j:2 * j + 2, nsl],
                       emit2, "2")

            def emit3(mi, nsl, pp):
                nn = nsl.stop - nsl.start
                sg = tpool.tile([128, 256], F16, tag="sg", name="sg")[:, :nn]
                nc.scalar.activation(sg, pp, Sigm,
                                     bias=c3_sb[:, mi:mi + 1], scale=1.0 / (SA * SW))
                nc.vector.tensor_tensor(sg, sg, wbc[:, nsl], op=AL.mult)
                nc.sync.dma_start(outb_d.ap()[mi * 128:(mi + 1) * 128, nsl], sg)

            _run_layer(nc, wpool, ps, V3_d.ap(), MO, KHP, ch23,
                       lambda j, hl, nsl: a2[:, 2 * j:2 * j + 2, nsl],
                       emit3, "3")
    nc.compile()
    return nc


def _build_C():
    """Shared expert layers 2+3 (with a2 hi/lo split) + final combine."""
    nc = bacc.Bacc("TRN2", target_bir_lowering=False, debug=False,
                   num_devices=N_CORES)
    a1_d = nc.dram_tensor("a1s", [TOK // 256, 128, MH, 256], F8,
                          kind="ExternalInput")
    V2_d = nc.dram_tensor("V2s", [MH, 128, KHP, 2, 2, 128], F8, kind="ExternalInput")
    V3_d = nc.dram_tensor("V3s", [MO, 128, KHP, 2, 2, 128], F8, kind="ExternalInput")
    C2_d = nc.dram_tensor("C2s", [128, MH], F32, kind="ExternalInput")
    C3_d = nc.dram_tensor("C3s", [128, MO], F32, kind="ExternalInput")
    c1t_d = nc.dram_tensor("cont1", [O, TOK], F16, kind="ExternalInput")
    c2t_d = nc.dram_tensor("cont2", [O, TOK], F16, kind="ExternalInput")
    out_d = nc.dram_tensor("out", [O, TOK], F32, kind="ExternalOutput")

    with tile.TileContext(nc) as tc:
        with tc.tile_pool(name="const", bufs=1) as cpool, \
             tc.tile_pool(name="acts", bufs=1) as apool, \
             tc.tile_pool(name="wts", bufs=4) as wpool, \
             tc.tile_pool(name="tmp", bufs=6) as tpool, \
             tc.tile_pool(name="ps", bufs=8, space="PSUM") as ps:
            a1 = cpool.tile([128, MH, TOK], F8, name="a1")
            for b in range(TOK // 256):
                nc.sync.dma_start(a1[:, :, b * 256:(b + 1) * 256], a1_d.ap()[b])
            c2_sb = cpool.tile([128, MH], F32, name="c2_sb")
            nc.sync.dma_start(c2_sb, C2_d.ap())
            c3_sb = cpool.tile([128, MO], F32, name="c3_sb")
            nc.sync.dma_start(c3_sb, C3_d.ap())

            a2hl = apool.tile([128, KH, 2, TOK], F8, name="a2hl")

            def emit2(mi, nsl, pp):
                nn = nsl.stop - nsl.start
                t32 = tpool.tile([128, 256], F32, tag="t32", name="t32")[:, :nn]
                nc.scalar.activation(t32, pp, Relu,
                                     bias=c2_sb[:, mi:mi + 1], scale=1.0 / SW)
                nc.vector.tensor_copy(a2hl[:, mi, 0, nsl], t32)
                nc.vector.tensor_tensor(a2hl[:, mi, 1, nsl], t32,
                                        a2hl[:, mi, 0, nsl], op=AL.subtract)

            _run_layer(nc, wpool, ps, V2_d.ap(), MH, KHP,
                       [(0, TOK, "ws")],
                       lambda j, hl, nsl: a1[:, 2 * j:2 * j + 2, nsl],
                       emit2, "2")

            def emit3(mi, nsl, pp):
                nn = nsl.stop - nsl.start
                sg = tpool.tile([128, 256], F16, tag="sg", name="sg")[:, :nn]
                nc.scalar.activation(sg, pp, Sigm,
                                     bias=c3_sb[:, mi:mi + 1], scale=1.0 / (SA * SW))
                ct1 = tpool.tile([128, 256], F16, tag="ct1", name="ct1")[:, :nn]
                nc.sync.dma_start(ct1, c1t_d.ap()[mi * 128:(mi + 1) * 128, nsl])
                ct2 = tpool.tile([128, 256], F16, tag="ct2", name="ct2")[:, :nn]
                nc.sync.dma_start(ct2, c2t_d.ap()[mi * 128:(mi + 1) * 128, nsl])
                s1 = tpool.tile([128, 256], F16, tag="s1", name="s1")[:, :nn]
                nc.vector.tensor_tensor(s1, sg, ct1, op=AL.add)
                so = tpool.tile([128, 256], F32, tag="so", name="so")[:, :nn]
                nc.vector.tensor_tensor(so, s1, ct2, op=AL.add)
                nc.sync.dma_start(out_d.ap()[mi * 128:(mi + 1) * 128, nsl], so)

            _run_layer(nc, wpool, ps, V3_d.ap(), MO, KHP,
                       [(0, TOK, "bs")],
                       lambda j, hl, nsl: a2hl[:, 2 * j:2 * j + 2, hl, nsl],
                       emit3, "3")
    nc.compile()
    return nc


# ------------------------------------------------------------------ host glue
def _r64(n):
    return max(64, (int(n) + 63) // 64 * 64)


def _route(wsum):
    """Per-expert tier column lists from device gate weights."""
    n = wsum.shape[0]
    e1 = np.argmax(wsum, axis=1)
    w1 = wsum[np.arange(n), e1]
    ws2 = wsum.copy()
    ws2[np.arange(n), e1] = 0.0
    e2 = np.argmax(ws2, axis=1)
    w2 = ws2[np.arange(n), e2]
    tiers = []  # per expert: (tokens, weights, is_first) ordered R,B,P
    nR, nRB, nT = [], [], []
    for e in range(E):
        f = e1 == e
        s = e2 == e
        tR = np.nonzero(f & (w1 > THR1))[0]
        tBf = np.nonzero(f & (w1 <= THR1))[0]
        tBs = np.nonzero(s & (w2 > THR2))[0]
        tP = np.nonzero(s & (w2 <= THR2))[0]
        toks = np.concatenate([tR, tBf, tBs, tP])
        wv = np.concatenate([w1[tR], w1[tBf], w2[tBs], w2[tP]])
        isf = np.concatenate([np.ones(len(tR) + len(tBf), bool),
                              np.zeros(len(tBs) + len(tP), bool)])
        tiers.append((toks, wv, isf))
        nR.append(len(tR))
        nRB.append(len(tR) + len(tBf) + len(tBs))
        nT.append(len(toks))
    # chunk-prefix capacities: tokens placed consecutively may only be
    # upgraded to a higher-precision mode, never downgraded
    capR = _r64(max(nR))
    capRB = max(capR, _r64(max(nRB)))
    CT = max(capRB, _r64(max(nT)))
    return tiers, (capR, capRB - capR, CT - capRB)


_CACHED = {}


def kernel(**inputs) -> np.ndarray:
    inp = {k: np.asarray(v) for k, v in inputs.items()}
    folded = _fold_params(inp)
    x = inp['x'].astype(np.float32)
    WgT = np.ascontiguousarray(inp['Wg'].T.astype(np.float32))
    sh = folded[E]

    # ---- launch A: gate + x split + shared L1 ----
    if "A" not in _CACHED:
        _CACHED["A"] = _build_A()
    ncA = _CACHED["A"]
    mapsA = []
    for c in range(N_CORES):
        xT = np.ascontiguousarray(x[c * TOK:(c + 1) * TOK].T)
        mapsA.append(dict(xT=xT, WgT=WgT, V1s=sh["V1"], C1s=sh["c1"]))
    resA = run_bass_kernel_spmd(ncA, mapsA, core_ids=list(range(N_CORES)))
    wsum = np.concatenate(
        [r["wsum"].reshape(128, GT, E).transpose(1, 0, 2).reshape(TOK, E)
         for r in resA.results], axis=0)
    xcat = np.concatenate([r["xhl"] for r in resA.results], axis=-1)
    a1s = [r["a1s"] for r in resA.results]

    # ---- host dispatch ----
    tiers, caps = _route(wsum)
    capR, capB, capP = caps
    CT = capR + capB + capP

    if _CACHED.get("B_caps") != caps:
        _CACHED["B"] = _build_B(caps)
        _CACHED["B_caps"] = caps
    ncB = _CACHED["B"]
    NBX = -(-CT // 256)
    mapsB = []
    colmaps = []
    for e in range(E):
        toks, wv, isf = tiers[e]
        m = len(toks)
        xg = np.zeros((128, KD, 2, NBX * 256), E4)
        xg[:, :, :, :m] = xcat[:, :, :, toks]
        wrow = np.zeros((CT,), np.float16)
        wrow[:m] = wv.astype(np.float16)
        colmaps.append((np.arange(m), toks, isf))
        fe = folded[e]
        mapsB.append(dict(
            xg=np.ascontiguousarray(
                xg.reshape(128, KD, 2, NBX, 256).transpose(3, 0, 1, 2, 4)),
            wrow=wrow, V1s=fe["V1"], V2s=fe["V2"], V3s=fe["V3"],
            C1s=fe["c1"], C2s=fe["c2"], C3s=fe["c3"]))
    resB = run_bass_kernel_spmd(ncB, mapsB, core_ids=list(range(N_CORES)))

    # ---- host combine alignment (column scatter, channel-major) ----
    cont1 = np.zeros((O, N_TOKENS), np.float16)
    cont2 = np.zeros((O, N_TOKENS), np.float16)
    for e in range(E):
        cols, toks, isf = colmaps[e]
        ob = resB.results[e]["outb"]
        cont1[:, toks[isf]] = ob[:, cols[isf]]
        cont2[:, toks[~isf]] = ob[:, cols[~isf]]

    # ---- launch C: shared L2+L3 + combine ----
    if "C" not in _CACHED:
        _CACHED["C"] = _build_C()
    ncC = _CACHED["C"]
    mapsC = []
    for c in range(N_CORES):
        sl = slice(c * TOK, (c + 1) * TOK)
        mapsC.append(dict(a1s=a1s[c], V2s=sh["V2"], V3s=sh["V3"],
                          C2s=sh["c2"], C3s=sh["c3"],
                          cont1=np.ascontiguousarray(cont1[:, sl]),
                          cont2=np.ascontiguousarray(cont2[:, sl])))
    resC = run_bass_kernel_spmd(ncC, mapsC, core_ids=list(range(N_CORES)))
    out = np.concatenate([np.ascontiguousarray(r["out"].T)
                          for r in resC.results], axis=0)

    _CACHED["timing"] = [(ncA, mapsA), (ncB, mapsB), (ncC, mapsC)]
    return out.astype(np.float32)


# revision 20
# speedup vs baseline: 1.0921x; 1.0921x over previous
"""DeepseekMoE Trainium2 kernel — routed 3-launch pipeline on 8 NeuronCores.

All FFN matmuls run as fp8(e4m3) DoubleRow tensor ops (0.5 cycles/row, 256-wide
contraction per instruction) with per-operand hi/lo residual splits choosing a
precision tier per (token, expert-slot):
  R  (320 cyc/tok): x hi/lo, W hi/lo, a1/a2 single fp8  — high combine weight
  B1 (256 cyc/tok): like R but x single fp8             — mid weight
  P8 (128 cyc/tok): everything single fp8               — low weight (w2<=0.35)
The shared expert runs scheme F (R plus an a2 hi/lo split).  Weight hi/lo
splits are host-side parameter preprocessing; the only data-dependent splits
(x, shared a2) are computed on device.

Launch A (data-parallel): fp32 gate (top-2 via sigmoid identity
  w1 = sigmoid(s1 - s2)), device x hi/lo split, shared-expert layer 1.
Launch B (expert-parallel, one expert per core): 3-layer FFN over
  host-gathered tokens in three tier chunks; outputs weighted fp16.
Launch C (data-parallel): shared layers 2+3 and final combine
  out = shared + cont1 + cont2.
Host code between launches only moves data (gather/scatter/layout); all
per-token arithmetic is on device.
"""
import numpy as np
import ml_dtypes
import concourse.mybir as mybir
import concourse.tile as tile
from concourse import bacc
from concourse.bass_utils import run_bass_kernel_spmd

F32 = mybir.dt.float32
F16 = mybir.dt.float16
F8 = mybir.dt.float8e4
E4 = ml_dtypes.float8_e4m3
DR = mybir.MatmulPerfMode.DoubleRow
AL = mybir.AluOpType
Relu = mybir.ActivationFunctionType.Relu
Sigm = mybir.ActivationFunctionType.Sigmoid

N_TOKENS, D, H, O, E = 8192, 1024, 2048, 1024, 8
N_CORES, TOK = 8, 1024
KD, KH = D // 128, H // 128          # contraction 128-blocks
KDP, KHP = KD // 2, KH // 2          # DoubleRow k-pairs
MH, MO = H // 128, O // 128          # output 128-tiles
GT = TOK // 128                      # gate token tiles per core
EPS = 1e-5
SW, SA = 32.0, 8.0                   # weight / activation fp8 storage scales
BIG = 1e30
THR1, THR2 = 0.6, 0.35               # tier thresholds on combine weight


# ---------------------------------------------------------------- host prep
def _wlayout(V):
    """V [K, M] fp32 -> [MT, 128, KP, 2, 2, 128] e4m3 hi/lo DoubleRow layout.
    k = j*256 + ksub*128 + p ; m = mi*128 + mm ; dim4 = (hi, lo)."""
    Kd, Md = V.shape
    KP, MT = Kd // 256, Md // 128
    s = (V * SW).astype(np.float32)
    hi = s.astype(E4)
    lo = (s - hi.astype(np.float32)).astype(E4)
    out = np.empty((MT, 128, KP, 2, 2, 128), E4)
    out[..., 0, :] = hi.reshape(KP, 2, 128, MT, 128).transpose(3, 2, 0, 1, 4)
    out[..., 1, :] = lo.reshape(KP, 2, 128, MT, 128).transpose(3, 2, 0, 1, 4)
    return np.ascontiguousarray(out)


def _fold_params(inp):
    """Fold eval-mode BN into weights; emit fp8 hi/lo layouts + scaled biases."""
    out = []
    for e in range(E + 1):
        if e < E:
            W1, b1 = inp['W1'][e], inp['b1'][e]
            g1, be1, m1, v1 = inp['g1'][e], inp['be1'][e], inp['m1'][e], inp['v1'][e]
            W2, b2 = inp['W2'][e], inp['b2'][e]
            g2, be2, m2, v2 = inp['g2'][e], inp['be2'][e], inp['m2'][e], inp['v2'][e]
            W3, b3 = inp['W3'][e], inp['b3'][e]
        else:
            W1, b1 = inp['sW1'], inp['sb1']
            g1, be1, m1, v1 = inp['sg1'], inp['sbe1'], inp['sm1'], inp['sv1']
            W2, b2 = inp['sW2'], inp['sb2']
            g2, be2, m2, v2 = inp['sg2'], inp['sbe2'], inp['sm2'], inp['sv2']
            W3, b3 = inp['sW3'], inp['sb3']
        s1 = g1 / np.sqrt(v1 + EPS); t1 = be1 - m1 * s1
        s2 = g2 / np.sqrt(v2 + EPS); t2 = be2 - m2 * s2
        V1 = W1.T.astype(np.float32)
        V2 = (s1[:, None] * W2.T).astype(np.float32)
        V3 = (s2[:, None] * W3.T).astype(np.float32)
        c1 = (SA * b1).astype(np.float32)
        c2 = (SA * (b2 + t1 @ W2.T)).astype(np.float32)
        c3 = (b3 + t2 @ W3.T).astype(np.float32)
        out.append(dict(
            V1=_wlayout(V1), V2=_wlayout(V2), V3=_wlayout(V3),
            c1=np.ascontiguousarray(c1.reshape(MH, 128).T),
            c2=np.ascontiguousarray(c2.reshape(MH, 128).T),
            c3=np.ascontiguousarray(c3.reshape(MO, 128).T)))
    return out


# ------------------------------------------------------------ layer builder
def _mm_group(nc, ps, wt, KP, mode, mov, nsl, emit, mi, tag):
    nn = nsl.stop - nsl.start
    pp = ps.tile([128, 512], F32, tag="ps", name=f"pp{tag}")[:, :nn]
    seq = []
    for j in range(KP):
        hi_st = wt[:, j, :, 0, :]
        lo_st = wt[:, j, :, 1, :]
        if mode == "p8":
            seq.append((hi_st, mov(j, 0, nsl)))
        elif mode == "ws":
            mh = mov(j, 0, nsl)
            seq.append((hi_st, mh))
            seq.append((lo_st, mh))
        else:  # both-split
            mh, ml = mov(j, 0, nsl), mov(j, 1, nsl)
            seq += [(hi_st, mh), (hi_st, ml), (lo_st, mh), (lo_st, ml)]
    for i, (st, mv) in enumerate(seq):
        nc.tensor.matmul(pp, st, mv, start=(i == 0),
                         stop=(i == len(seq) - 1), perf_mode=DR)
    emit(mi, nsl, pp)


def _slices(chunks):
    out = []
    for (off, size, mode) in chunks:
        s = 0
        while s < size:
            nn = min(256, size - s)
            out.append((slice(off + s, off + s + nn), mode))
            s += nn
    return out


def _run_layer(nc, wpool, ps, wdram, MT, KP, chunks, mov, emit, tag):
    """One FFN layer over token chunks (mi-outer, streaming weights).
    chunks: [(off, size, mode)]; mov(j, hl, nsl) -> [128, 2, nn] AP."""
    sls = _slices(chunks)
    for mi in range(MT):
        wt = wpool.tile([128, KP, 2, 2, 128], F8, tag="wt", name=f"wt{tag}")
        nc.sync.dma_start(wt, wdram[mi])
        for (nsl, mode) in sls:
            _mm_group(nc, ps, wt, KP, mode, mov, nsl, emit, mi, tag)


# ------------------------------------------------------------ kernel builders
def _build_A():
    """Gate (fp32) + x hi/lo split + shared-expert layer 1."""
    nc = bacc.Bacc("TRN2", target_bir_lowering=False, debug=False,
                   num_devices=N_CORES)
    xT_d = nc.dram_tensor("xT", [D, TOK], F32, kind="ExternalInput")
    wg_d = nc.dram_tensor("WgT", [D, E], F32, kind="ExternalInput")
    V1_d = nc.dram_tensor("V1s", [MH, 128, KDP, 2, 2, 128], F8, kind="ExternalInput")
    C1_d = nc.dram_tensor("C1s", [128, MH], F32, kind="ExternalInput")
    ws_d = nc.dram_tensor("wsum", [128, GT * E], F32, kind="ExternalOutput")
    xhl_d = nc.dram_tensor("xhl", [128, KD, 2, TOK], F8, kind="ExternalOutput")
    a1s_d = nc.dram_tensor("a1s", [TOK // 256, 128, MH, 256], F8,
                           kind="ExternalOutput")

    NB = TOK // 256
    with tile.TileContext(nc) as tc:
        with tc.tile_pool(name="const", bufs=1) as cpool, \
             tc.tile_pool(name="gate", bufs=1) as gpool, \
             tc.tile_pool(name="ps", bufs=8, space="PSUM") as ps:
            wg = cpool.tile([128, KD, E], F32, name="wg")
            nc.sync.dma_start(wg, wg_d.ap().rearrange("(k p) e -> p k e", p=128))
            c1_sb = cpool.tile([128, MH], F32, name="c1_sb")
            nc.sync.dma_start(c1_sb, C1_d.ap())
            # x loaded in 4 column blocks so the pipeline starts early
            x32 = cpool.tile([128, KD, TOK], F32, name="x32")
            for b in range(NB):
                csl = slice(b * 256, (b + 1) * 256)
                nc.sync.dma_start(
                    x32[:, :, csl],
                    xT_d.ap().rearrange("(k p) t -> p k t", p=128)[:, :, csl])
            # resident layer-1 weights (32 KiB/partition), two bulk loads
            wt1 = cpool.tile([128, MH, KDP, 2, 2, 128], F8, name="wt1")
            nc.sync.dma_start(wt1[:, :MH // 2], V1_d.ap().rearrange(
                "a p b c d e -> p a b c d e")[:, :MH // 2])
            nc.sync.dma_start(wt1[:, MH // 2:], V1_d.ap().rearrange(
                "a p b c d e -> p a b c d e")[:, MH // 2:])

            # x hi/lo split (column-blocked: Act does hi, DVE does lo)
            xhl = cpool.tile([128, KD, 2, TOK], F8, name="xhl")
            for b in range(NB):
                csl = slice(b * 256, (b + 1) * 256)
                for kb in range(KD):
                    nc.scalar.activation(xhl[:, kb, 0, csl], x32[:, kb, csl],
                                         mybir.ActivationFunctionType.Copy,
                                         bias=0.0, scale=1.0)
                    nc.vector.tensor_tensor(xhl[:, kb, 1, csl], x32[:, kb, csl],
                                            xhl[:, kb, 0, csl], op=AL.subtract)
            nc.sync.dma_start(xhl_d.ap(), xhl)

            # gate scores -> batched top-2 sigmoid weights
            sg = gpool.tile([128, GT, E], F32, name="sg")
            for ti in range(GT):
                tsl = slice(ti * 128, (ti + 1) * 128)
                pg = ps.tile([128, 512], F32, tag="ps", name="pg")[:, :E]
                for kb in range(KD):
                    nc.tensor.matmul(pg, x32[:, kb, tsl], wg[:, kb],
                                     start=(kb == 0), stop=(kb == KD - 1))
                nc.vector.tensor_copy(sg[:, ti], pg)
            m1 = gpool.tile([128, GT, 1], F32, name="m1")
            nc.vector.tensor_reduce(m1, sg, axis=mybir.AxisListType.X, op=AL.max)
            msk1 = gpool.tile([128, GT, E], F32, name="msk1")
            nc.vector.tensor_tensor(msk1, sg, m1.to_broadcast((128, GT, E)),
                                    op=AL.is_equal)
            pen = gpool.tile([128, GT, E], F32, name="pen")
            nc.vector.tensor_scalar_mul(pen, msk1, -BIG)
            nc.vector.tensor_tensor(pen, sg, pen, op=AL.add)
            m2 = gpool.tile([128, GT, 1], F32, name="m2")
            nc.vector.tensor_reduce(m2, pen, axis=mybir.AxisListType.X, op=AL.max)
            dm = gpool.tile([128, GT, 1], F32, name="dm")
            nc.vector.tensor_tensor(dm, m1, m2, op=AL.subtract)
            w1 = gpool.tile([128, GT, 1], F32, name="w1")
            nc.scalar.activation(w1, dm, Sigm, bias=0.0, scale=1.0)
            msk2 = gpool.tile([128, GT, E], F32, name="msk2")
            nc.vector.tensor_tensor(msk2, pen, m2.to_broadcast((128, GT, E)),
                                    op=AL.is_equal)
            t1 = gpool.tile([128, GT, E], F32, name="t1")
            nc.vector.tensor_tensor(t1, msk1, w1.to_broadcast((128, GT, E)),
                                    op=AL.mult)
            w2 = gpool.tile([128, GT, 1], F32, name="w2")
            nc.vector.tensor_scalar(w2, w1, -1.0, 1.0, op0=AL.mult, op1=AL.add)
            t2 = gpool.tile([128, GT, E], F32, name="t2")
            nc.vector.tensor_tensor(t2, msk2, w2.to_broadcast((128, GT, E)),
                                    op=AL.mult)
            wsm = gpool.tile([128, GT, E], F32, name="wsm")
            nc.vector.tensor_tensor(wsm, t1, t2, op=AL.add)
            nc.sync.dma_start(ws_d.ap(), wsm.rearrange("p a b -> p (a b)"))

            # shared expert layer 1 (both-split), block-outer so output
            # blocks stream out as they complete (block-major staging keeps
            # the outbound DMA descriptors 4 KiB-contiguous)
            a1s = cpool.tile([128, NB, MH, 256], F8, name="a1s")

            def emit1(mi, nsl, pp):
                nc.scalar.activation(a1s[:, nsl.start // 256, mi, :], pp, Relu,
                                     bias=c1_sb[:, mi:mi + 1], scale=SA / SW)

            mov1 = lambda j, hl, nsl: xhl[:, 2 * j:2 * j + 2, hl, nsl]
            for b in range(NB):
                csl = slice(b * 256, (b + 1) * 256)
                for mi in range(MH):
                    _mm_group(nc, ps, wt1[:, mi], KDP, "bs", mov1, csl,
                              emit1, mi, "1")
                nc.sync.dma_start(a1s_d.ap()[b], a1s[:, b])
    nc.compile()
    return nc


def _build_B(caps):
    """One expert per core over gathered tokens in tier chunks (R, B1, P8)."""
    capR, capB, capP = caps
    CT = capR + capB + capP
    nc = bacc.Bacc("TRN2", target_bir_lowering=False, debug=False,
                   num_devices=N_CORES)
    xg_d = nc.dram_tensor("xg", [2, 128, KD, CT], F8, kind="ExternalInput")
    wr_d = nc.dram_tensor("wrow", [CT], F16, kind="ExternalInput")
    V1_d = nc.dram_tensor("V1s", [MH, 128, KDP, 2, 2, 128], F8, kind="ExternalInput")
    V2_d = nc.dram_tensor("V2s", [MH, 128, KHP, 2, 2, 128], F8, kind="ExternalInput")
    V3_d = nc.dram_tensor("V3s", [MO, 128, KHP, 2, 2, 128], F8, kind="ExternalInput")
    C1_d = nc.dram_tensor("C1s", [128, MH], F32, kind="ExternalInput")
    C2_d = nc.dram_tensor("C2s", [128, MH], F32, kind="ExternalInput")
    C3_d = nc.dram_tensor("C3s", [128, MO], F32, kind="ExternalInput")
    outb_d = nc.dram_tensor("outb", [O, CT], F16, kind="ExternalOutput")

    # B1/P8 chunks first: they only need the x hi plane, so compute starts
    # before the lo plane lands (only the R chunk reads it)
    ch1 = [(capR, capB, "ws"), (capR + capB, capP, "p8"), (0, capR, "bs")]
    ch23 = [(capR, capB, "ws"), (capR + capB, capP, "p8"), (0, capR, "ws")]

    with tile.TileContext(nc) as tc:
        with tc.tile_pool(name="const", bufs=1) as cpool, \
             tc.tile_pool(name="acts", bufs=1) as apool, \
             tc.tile_pool(name="wts", bufs=4) as wpool, \
             tc.tile_pool(name="tmp", bufs=4) as tpool, \
             tc.tile_pool(name="ps", bufs=8, space="PSUM") as ps:
            xg = cpool.tile([128, 2, KD, CT], F8, name="xg")
            nc.sync.dma_start(xg[:, 0], xg_d.ap()[0])
            wbc = cpool.tile([128, CT], F16, name="wbc")
            nc.sync.dma_start(wbc, wr_d.ap()[None, :].to_broadcast((128, CT)))
            c1_sb = cpool.tile([128, MH], F32, name="c1_sb")
            nc.sync.dma_start(c1_sb, C1_d.ap())
            c2_sb = cpool.tile([128, MH], F32, name="c2_sb")
            nc.sync.dma_start(c2_sb, C2_d.ap())
            c3_sb = cpool.tile([128, MO], F32, name="c3_sb")
            nc.sync.dma_start(c3_sb, C3_d.ap())
            nc.sync.dma_start(xg[:, 1], xg_d.ap()[1])

            a1 = apool.tile([128, KH, CT], F8, name="a1")
            a2 = apool.tile([128, KH, CT], F8, name="a2")

            def emit1(mi, nsl, pp):
                nc.scalar.activation(a1[:, mi, nsl], pp, Relu,
                                     bias=c1_sb[:, mi:mi + 1], scale=SA / SW)

            _run_layer(nc, wpool, ps, V1_d.ap(), MH, KDP, ch1,
                       lambda j, hl, nsl: xg[:, hl, 2 * j:2 * j + 2, nsl],
                       emit1, "1")

            def emit2(mi, nsl, pp):
                nc.scalar.activation(a2[:, mi, nsl], pp, Relu,
                                     bias=c2_sb[:, mi:mi + 1], scale=1.0 / SW)

            _run_layer(nc, wpool, ps, V2_d.ap(), MH, KHP, ch23,
                       lambda j, hl, nsl: a1[:, 2 * j:2 * j + 2, nsl],
                       emit2, "2")

            # stage per-mi output rows, one bulk DMA per mi
            sls3 = _slices(ch23)
            mov3 = lambda j, hl, nsl: a2[:, 2 * j:2 * j + 2, nsl]
            for mi in range(MO):
                wt = wpool.tile([128, KHP, 2, 2, 128], F8, tag="wt", name="wt3")
                nc.sync.dma_start(wt, V3_d.ap()[mi])
                ob = tpool.tile([128, CT], F16, tag="ob", name="ob")

                def emit3(_mi, nsl, pp, ob=ob):
                    sg = ob[:, nsl]
                    nc.scalar.activation(sg, pp, Sigm,
                                         bias=c3_sb[:, mi:mi + 1],
                                         scale=1.0 / (SA * SW))
                    nc.vector.tensor_tensor(sg, sg, wbc[:, nsl], op=AL.mult)

                for (nsl, mode) in sls3:
                    _mm_group(nc, ps, wt, KHP, mode, mov3, nsl, emit3, mi, "3")
                nc.sync.dma_start(outb_d.ap()[mi * 128:(mi + 1) * 128, :], ob)
    nc.compile()
    return nc


def _build_C():
    """Shared expert layers 2+3 (with a2 hi/lo split) + final combine."""
    nc = bacc.Bacc("TRN2", target_bir_lowering=False, debug=False,
                   num_devices=N_CORES)
    a1_d = nc.dram_tensor("a1s", [TOK // 256, 128, MH, 256], F8,
                          kind="ExternalInput")
    V2_d = nc.dram_tensor("V2s", [MH, 128, KHP, 2, 2, 128], F8, kind="ExternalInput")
    V3_d = nc.dram_tensor("V3s", [MO, 128, KHP, 2, 2, 128], F8, kind="ExternalInput")
    C2_d = nc.dram_tensor("C2s", [128, MH], F32, kind="ExternalInput")
    C3_d = nc.dram_tensor("C3s", [128, MO], F32, kind="ExternalInput")
    c1t_d = nc.dram_tensor("cont1", [O, TOK], F16, kind="ExternalInput")
    c2t_d = nc.dram_tensor("cont2", [O, TOK], F16, kind="ExternalInput")
    out_d = nc.dram_tensor("out", [O, TOK], F32, kind="ExternalOutput")

    NB = TOK // 256
    with tile.TileContext(nc) as tc:
        with tc.tile_pool(name="const", bufs=1) as cpool, \
             tc.tile_pool(name="acts", bufs=1) as apool, \
             tc.tile_pool(name="wts", bufs=4) as wpool, \
             tc.tile_pool(name="tmp", bufs=6) as tpool, \
             tc.tile_pool(name="ps", bufs=8, space="PSUM") as ps:
            a1 = cpool.tile([128, NB, MH, 256], F8, name="a1")
            nc.sync.dma_start(a1, a1_d.ap().rearrange("b p m t -> p b m t"))
            c2_sb = cpool.tile([128, MH], F32, name="c2_sb")
            nc.sync.dma_start(c2_sb, C2_d.ap())
            c3_sb = cpool.tile([128, MO], F32, name="c3_sb")
            nc.sync.dma_start(c3_sb, C3_d.ap())
            # resident layer-2 weights so layer 2 can run block-outer and
            # layer 3 starts as soon as the first a2 block completes
            wt2 = cpool.tile([128, MH, KHP, 2, 2, 128], F8, name="wt2")
            for mi in range(MH):
                nc.sync.dma_start(wt2[:, mi], V2_d.ap()[mi])

            a2hl = apool.tile([128, KH, 2, TOK], F8, name="a2hl")

            def emit2(mi, nsl, pp):
                nn = nsl.stop - nsl.start
                t32 = tpool.tile([128, 256], F32, tag="t32", name="t32")[:, :nn]
                nc.scalar.activation(t32, pp, Relu,
                                     bias=c2_sb[:, mi:mi + 1], scale=1.0 / SW)
                nc.vector.tensor_copy(a2hl[:, mi, 0, nsl], t32)
                nc.vector.tensor_tensor(a2hl[:, mi, 1, nsl], t32,
                                        a2hl[:, mi, 0, nsl], op=AL.subtract)

            mov2 = lambda j, hl, nsl: a1[:, nsl.start // 256, 2 * j:2 * j + 2, :]
            for b in range(NB):
                csl = slice(b * 256, (b + 1) * 256)
                for mi in range(MH):
                    _mm_group(nc, ps, wt2[:, mi], KHP, "ws", mov2, csl,
                              emit2, mi, "2")

            mov3 = lambda j, hl, nsl: a2hl[:, 2 * j:2 * j + 2, hl, nsl]
            for mi in range(MO):
                wt = wpool.tile([128, KHP, 2, 2, 128], F8, tag="wt", name="wt3")
                nc.sync.dma_start(wt, V3_d.ap()[mi])
                ct1 = tpool.tile([128, TOK], F16, tag="ct1", name="ct1")
                nc.sync.dma_start(ct1, c1t_d.ap()[mi * 128:(mi + 1) * 128])
                ct2 = tpool.tile([128, TOK], F16, tag="ct2", name="ct2")
                nc.sync.dma_start(ct2, c2t_d.ap()[mi * 128:(mi + 1) * 128])
                ob = tpool.tile([128, TOK], F32, tag="ob", name="ob")

                def emit3(_mi, nsl, pp, ct1=ct1, ct2=ct2, ob=ob, mi=mi):
                    sg = tpool.tile([128, 256], F16, tag="sg", name="sg")
                    nc.scalar.activation(sg, pp, Sigm,
                                         bias=c3_sb[:, mi:mi + 1],
                                         scale=1.0 / (SA * SW))
                    s1 = tpool.tile([128, 256], F16, tag="s1", name="s1")
                    nc.vector.tensor_tensor(s1, sg, ct1[:, nsl], op=AL.add)
                    nc.vector.tensor_tensor(ob[:, nsl], s1, ct2[:, nsl], op=AL.add)

                for b in range(NB):
                    csl = slice(b * 256, (b + 1) * 256)
                    _mm_group(nc, ps, wt, KHP, "bs", mov3, csl, emit3, mi, "3")
                nc.sync.dma_start(out_d.ap()[mi * 128:(mi + 1) * 128, :], ob)
    nc.compile()
    return nc


# ------------------------------------------------------------------ host glue
def _r64(n):
    return max(64, (int(n) + 63) // 64 * 64)


def _route(wsum):
    """Per-expert tier column lists from device gate weights."""
    n = wsum.shape[0]
    e1 = np.argmax(wsum, axis=1)
    w1 = wsum[np.arange(n), e1]
    ws2 = wsum.copy()
    ws2[np.arange(n), e1] = 0.0
    e2 = np.argmax(ws2, axis=1)
    w2 = ws2[np.arange(n), e2]
    tiers = []  # per expert: (tokens, weights, is_first) ordered R,B,P
    nR, nRB, nT = [], [], []
    for e in range(E):
        f = e1 == e
        s = e2 == e
        tR = np.nonzero(f & (w1 > THR1))[0]
        tBf = np.nonzero(f & (w1 <= THR1))[0]
        tBs = np.nonzero(s & (w2 > THR2))[0]
        tP = np.nonzero(s & (w2 <= THR2))[0]
        toks = np.concatenate([tR, tBf, tBs, tP])
        wv = np.concatenate([w1[tR], w1[tBf], w2[tBs], w2[tP]])
        isf = np.concatenate([np.ones(len(tR) + len(tBf), bool),
                              np.zeros(len(tBs) + len(tP), bool)])
        tiers.append((toks, wv, isf))
        nR.append(len(tR))
        nRB.append(len(tR) + len(tBf) + len(tBs))
        nT.append(len(toks))
    # chunk-prefix capacities: tokens placed consecutively may only be
    # upgraded to a higher-precision mode, never downgraded
    capR = _r64(max(nR))
    capRB = max(capR, _r64(max(nRB)))
    CT = max(capRB, _r64(max(nT)))
    return tiers, (capR, capRB - capR, CT - capRB)


_CACHED = {}


def kernel(**inputs) -> np.ndarray:
    inp = {k: np.asarray(v) for k, v in inputs.items()}
    folded = _fold_params(inp)
    x = inp['x'].astype(np.float32)
    WgT = np.ascontiguousarray(inp['Wg'].T.astype(np.float32))
    sh = folded[E]

    # ---- launch A: gate + x split + shared L1 ----
    if "A" not in _CACHED:
        _CACHED["A"] = _build_A()
    ncA = _CACHED["A"]
    mapsA = []
    for c in range(N_CORES):
        xT = np.ascontiguousarray(x[c * TOK:(c + 1) * TOK].T)
        mapsA.append(dict(xT=xT, WgT=WgT, V1s=sh["V1"], C1s=sh["c1"]))
    resA = run_bass_kernel_spmd(ncA, mapsA, core_ids=list(range(N_CORES)))
    wsum = np.concatenate(
        [r["wsum"].reshape(128, GT, E).transpose(1, 0, 2).reshape(TOK, E)
         for r in resA.results], axis=0)
    xcat = np.concatenate([r["xhl"] for r in resA.results], axis=-1)
    a1s = [r["a1s"] for r in resA.results]

    # ---- host dispatch ----
    tiers, caps = _route(wsum)
    capR, capB, capP = caps
    CT = capR + capB + capP

    if _CACHED.get("B_caps") != caps:
        _CACHED["B"] = _build_B(caps)
        _CACHED["B_caps"] = caps
    ncB = _CACHED["B"]
    mapsB = []
    colmaps = []
    for e in range(E):
        toks, wv, isf = tiers[e]
        m = len(toks)
        xg = np.zeros((2, 128, KD, CT), E4)
        xg[0, :, :, :m] = xcat[:, :, 0, toks]
        xg[1, :, :, :m] = xcat[:, :, 1, toks]
        wrow = np.zeros((CT,), np.float16)
        wrow[:m] = wv.astype(np.float16)
        colmaps.append((np.arange(m), toks, isf))
        fe = folded[e]
        mapsB.append(dict(
            xg=xg, wrow=wrow, V1s=fe["V1"], V2s=fe["V2"], V3s=fe["V3"],
            C1s=fe["c1"], C2s=fe["c2"], C3s=fe["c3"]))
    resB = run_bass_kernel_spmd(ncB, mapsB, core_ids=list(range(N_CORES)))

    # ---- host combine alignment (column scatter, channel-major) ----
    cont1 = np.zeros((O, N_TOKENS), np.float16)
    cont2 = np.zeros((O, N_TOKENS), np.float16)
    for e in range(E):
        cols, toks, isf = colmaps[e]
        ob = resB.results[e]["outb"]
        cont1[:, toks[isf]] = ob[:, cols[isf]]
        cont2[:, toks[~isf]] = ob[:, cols[~isf]]

    # ---- launch C: shared L2+L3 + combine ----
    if "C" not in _CACHED:
        _CACHED["C"] = _build_C()
    ncC = _CACHED["C"]
    mapsC = []
    for c in range(N_CORES):
        sl = slice(c * TOK, (c + 1) * TOK)
        mapsC.append(dict(a1s=a1s[c], V2s=sh["V2"], V3s=sh["V3"],
                          C2s=sh["c2"], C3s=sh["c3"],
                          cont1=np.ascontiguousarray(cont1[:, sl]),
                          cont2=np.ascontiguousarray(cont2[:, sl])))
    resC = run_bass_kernel_spmd(ncC, mapsC, core_ids=list(range(N_CORES)))
    out = np.concatenate([np.ascontiguousarray(r["out"].T)
                          for r in resC.results], axis=0)

    _CACHED["timing"] = [(ncA, mapsA), (ncB, mapsB), (ncC, mapsC)]
    return out.astype(np.float32)


# revision 23
# speedup vs baseline: 1.1200x; 1.0255x over previous
"""DeepseekMoE Trainium2 kernel — routed 3-launch pipeline on 8 NeuronCores.

All FFN matmuls run as fp8(e4m3) DoubleRow tensor ops (0.5 cycles/row, 256-wide
contraction per instruction) with per-operand hi/lo residual splits choosing a
precision tier per (token, expert-slot):
  R  (320 cyc/tok): x hi/lo, W hi/lo, a1/a2 single fp8  — high combine weight
  B1 (256 cyc/tok): like R but x single fp8             — mid weight
  P8 (128 cyc/tok): everything single fp8               — low weight (w2<=0.35)
The shared expert runs scheme F (R plus an a2 hi/lo split).  Weight hi/lo
splits are host-side parameter preprocessing; the only data-dependent splits
(x, shared a2) are computed on device.

Launch A (data-parallel): fp32 gate (top-2 via sigmoid identity
  w1 = sigmoid(s1 - s2)), device x hi/lo split, shared-expert layer 1.
Launch B (expert-parallel, one expert per core): 3-layer FFN over
  host-gathered tokens in three tier chunks; outputs weighted fp16.
Launch C (data-parallel): shared layers 2+3 and final combine
  out = shared + cont1 + cont2.
Host code between launches only moves data (gather/scatter/layout); all
per-token arithmetic is on device.
"""
import numpy as np
import ml_dtypes
import concourse.mybir as mybir
import concourse.tile as tile
from concourse import bacc
from concourse.bass_utils import run_bass_kernel_spmd

F32 = mybir.dt.float32
F16 = mybir.dt.float16
F8 = mybir.dt.float8e4
E4 = ml_dtypes.float8_e4m3
DR = mybir.MatmulPerfMode.DoubleRow
AL = mybir.AluOpType
Relu = mybir.ActivationFunctionType.Relu
Sigm = mybir.ActivationFunctionType.Sigmoid

N_TOKENS, D, H, O, E = 8192, 1024, 2048, 1024, 8
N_CORES, TOK = 8, 1024
KD, KH = D // 128, H // 128          # contraction 128-blocks
KDP, KHP = KD // 2, KH // 2          # DoubleRow k-pairs
MH, MO = H // 128, O // 128          # output 128-tiles
GT = TOK // 128                      # gate token tiles per core
EPS = 1e-5
SW, SA = 32.0, 8.0                   # weight / activation fp8 storage scales
BIG = 1e30
THR1, THR2 = 0.6, 0.35               # tier thresholds on combine weight


# ---------------------------------------------------------------- host prep
def _wlayout(V):
    """V [K, M] fp32 -> [MT, 128, KP, 2, 2, 128] e4m3 hi/lo DoubleRow layout.
    k = j*256 + ksub*128 + p ; m = mi*128 + mm ; dim4 = (hi, lo)."""
    Kd, Md = V.shape
    KP, MT = Kd // 256, Md // 128
    s = (V * SW).astype(np.float32)
    hi = s.astype(E4)
    lo = (s - hi.astype(np.float32)).astype(E4)
    out = np.empty((MT, 128, KP, 2, 2, 128), E4)
    out[..., 0, :] = hi.reshape(KP, 2, 128, MT, 128).transpose(3, 2, 0, 1, 4)
    out[..., 1, :] = lo.reshape(KP, 2, 128, MT, 128).transpose(3, 2, 0, 1, 4)
    return np.ascontiguousarray(out)


def _fold_params(inp):
    """Fold eval-mode BN into weights; emit fp8 hi/lo layouts + scaled biases."""
    out = []
    for e in range(E + 1):
        if e < E:
            W1, b1 = inp['W1'][e], inp['b1'][e]
            g1, be1, m1, v1 = inp['g1'][e], inp['be1'][e], inp['m1'][e], inp['v1'][e]
            W2, b2 = inp['W2'][e], inp['b2'][e]
            g2, be2, m2, v2 = inp['g2'][e], inp['be2'][e], inp['m2'][e], inp['v2'][e]
            W3, b3 = inp['W3'][e], inp['b3'][e]
        else:
            W1, b1 = inp['sW1'], inp['sb1']
            g1, be1, m1, v1 = inp['sg1'], inp['sbe1'], inp['sm1'], inp['sv1']
            W2, b2 = inp['sW2'], inp['sb2']
            g2, be2, m2, v2 = inp['sg2'], inp['sbe2'], inp['sm2'], inp['sv2']
            W3, b3 = inp['sW3'], inp['sb3']
        s1 = g1 / np.sqrt(v1 + EPS); t1 = be1 - m1 * s1
        s2 = g2 / np.sqrt(v2 + EPS); t2 = be2 - m2 * s2
        V1 = W1.T.astype(np.float32)
        V2 = (s1[:, None] * W2.T).astype(np.float32)
        V3 = (s2[:, None] * W3.T).astype(np.float32)
        c1 = (SA * b1).astype(np.float32)
        c2 = (SA * (b2 + t1 @ W2.T)).astype(np.float32)
        c3 = (b3 + t2 @ W3.T).astype(np.float32)
        out.append(dict(
            V1=_wlayout(V1), V2=_wlayout(V2), V3=_wlayout(V3),
            c1=np.ascontiguousarray(c1.reshape(MH, 128).T),
            c2=np.ascontiguousarray(c2.reshape(MH, 128).T),
            c3=np.ascontiguousarray(c3.reshape(MO, 128).T)))
    return out


# ------------------------------------------------------------ layer builder
def _mm_group(nc, ps, wt, KP, mode, mov, nsl, emit, mi, tag):
    nn = nsl.stop - nsl.start
    pp = ps.tile([128, 512], F32, tag="ps", name=f"pp{tag}")[:, :nn]
    seq = []
    for j in range(KP):
        hi_st = wt[:, j, :, 0, :]
        lo_st = wt[:, j, :, 1, :]
        if mode == "p8":
            seq.append((hi_st, mov(j, 0, nsl)))
        elif mode == "ws":
            mh = mov(j, 0, nsl)
            seq.append((hi_st, mh))
            seq.append((lo_st, mh))
        else:  # both-split
            mh, ml = mov(j, 0, nsl), mov(j, 1, nsl)
            seq += [(hi_st, mh), (hi_st, ml), (lo_st, mh), (lo_st, ml)]
    for i, (st, mv) in enumerate(seq):
        nc.tensor.matmul(pp, st, mv, start=(i == 0),
                         stop=(i == len(seq) - 1), perf_mode=DR)
    emit(mi, nsl, pp)


def _slices(chunks):
    out = []
    for (off, size, mode) in chunks:
        s = 0
        while s < size:
            nn = min(256, size - s)
            out.append((slice(off + s, off + s + nn), mode))
            s += nn
    return out


def _run_layer(nc, wpool, ps, wdram, MT, KP, chunks, mov, emit, tag):
    """One FFN layer over token chunks (mi-outer, streaming weights).
    chunks: [(off, size, mode)]; mov(j, hl, nsl) -> [128, 2, nn] AP."""
    sls = _slices(chunks)
    for mi in range(MT):
        wt = wpool.tile([128, KP, 2, 2, 128], F8, tag="wt", name=f"wt{tag}")
        nc.sync.dma_start(wt, wdram[mi])
        for (nsl, mode) in sls:
            _mm_group(nc, ps, wt, KP, mode, mov, nsl, emit, mi, tag)


# ------------------------------------------------------------ kernel builders
def _build_A():
    """Gate (fp32) + x hi/lo split + shared-expert layer 1."""
    nc = bacc.Bacc("TRN2", target_bir_lowering=False, debug=False,
                   num_devices=N_CORES)
    xT_d = nc.dram_tensor("xT", [D, TOK], F32, kind="ExternalInput")
    wg_d = nc.dram_tensor("WgT", [D, E], F32, kind="ExternalInput")
    V1_d = nc.dram_tensor("V1s", [MH, 128, KDP, 2, 2, 128], F8, kind="ExternalInput")
    C1_d = nc.dram_tensor("C1s", [128, MH], F32, kind="ExternalInput")
    ws_d = nc.dram_tensor("wsum", [128, GT * E], F32, kind="ExternalOutput")
    xhl_d = nc.dram_tensor("xhl", [128, KD, 2, TOK], F8, kind="ExternalOutput")
    a1s_d = nc.dram_tensor("a1s", [TOK // 256, 128, MH, 256], F8,
                           kind="ExternalOutput")

    NB = TOK // 256
    with tile.TileContext(nc) as tc:
        with tc.tile_pool(name="const", bufs=1) as cpool, \
             tc.tile_pool(name="gate", bufs=1) as gpool, \
             tc.tile_pool(name="ps", bufs=8, space="PSUM") as ps:
            wg = cpool.tile([128, KD, E], F32, name="wg")
            nc.sync.dma_start(wg, wg_d.ap().rearrange("(k p) e -> p k e", p=128))
            c1_sb = cpool.tile([128, MH], F32, name="c1_sb")
            nc.sync.dma_start(c1_sb, C1_d.ap())
            # criticality-ordered loads: x block 0, first weights, then the
            # rest interleaved so layer 1 streams without stalls
            x32 = cpool.tile([128, KD, TOK], F32, name="x32")
            wt1 = cpool.tile([128, MH, KDP, 2, 2, 128], F8, name="wt1")
            xT_ap = xT_d.ap().rearrange("(k p) t -> p k t", p=128)
            wt_ap = V1_d.ap().rearrange("a p b c d e -> p a b c d e")

            def ldx(b):
                csl = slice(b * 256, (b + 1) * 256)
                nc.sync.dma_start(x32[:, :, csl], xT_ap[:, :, csl])

            def ldw(m0, m1_):
                nc.sync.dma_start(wt1[:, m0:m1_], wt_ap[:, m0:m1_])

            ldx(0); ldw(0, 4); ldw(4, 8); ldw(8, 12); ldx(1); ldw(12, 16)
            ldx(2); ldx(3)

            # x hi/lo split (column-blocked: Pool does hi, DVE does lo)
            xhl = cpool.tile([128, KD, 2, TOK], F8, name="xhl")
            for b in range(NB):
                csl = slice(b * 256, (b + 1) * 256)
                for kb in range(KD):
                    nc.gpsimd.tensor_copy(xhl[:, kb, 0, csl], x32[:, kb, csl])
                    nc.vector.tensor_tensor(xhl[:, kb, 1, csl], x32[:, kb, csl],
                                            xhl[:, kb, 0, csl], op=AL.subtract)
            nc.sync.dma_start(xhl_d.ap(), xhl)

            # shared expert layer 1 (both-split), block-outer so output
            # blocks stream out as they complete (block-major staging keeps
            # the outbound DMA descriptors 4 KiB-contiguous)
            a1s = cpool.tile([128, NB, MH, 256], F8, name="a1s")

            def emit1(mi, nsl, pp):
                nc.scalar.activation(a1s[:, nsl.start // 256, mi, :], pp, Relu,
                                     bias=c1_sb[:, mi:mi + 1], scale=SA / SW)

            mov1 = lambda j, hl, nsl: xhl[:, 2 * j:2 * j + 2, hl, nsl]
            for b in range(NB):
                csl = slice(b * 256, (b + 1) * 256)
                for mi in range(MH):
                    _mm_group(nc, ps, wt1[:, mi], KDP, "bs", mov1, csl,
                              emit1, mi, "1")
                nc.sync.dma_start(a1s_d.ap()[b], a1s[:, b])

            # gate scores -> batched top-2 sigmoid weights (emitted last:
            # nothing on-device consumes wsum, so it fills PE idle slots)
            sg = gpool.tile([128, GT, E], F32, name="sg")
            for ti in range(GT):
                tsl = slice(ti * 128, (ti + 1) * 128)
                pg = ps.tile([128, 512], F32, tag="ps", name="pg")[:, :E]
                for kb in range(KD):
                    nc.tensor.matmul(pg, x32[:, kb, tsl], wg[:, kb],
                                     start=(kb == 0), stop=(kb == KD - 1))
                nc.vector.tensor_copy(sg[:, ti], pg)
            m1 = gpool.tile([128, GT, 1], F32, name="m1")
            nc.vector.tensor_reduce(m1, sg, axis=mybir.AxisListType.X, op=AL.max)
            msk1 = gpool.tile([128, GT, E], F32, name="msk1")
            nc.vector.tensor_tensor(msk1, sg, m1.to_broadcast((128, GT, E)),
                                    op=AL.is_equal)
            pen = gpool.tile([128, GT, E], F32, name="pen")
            nc.vector.tensor_scalar_mul(pen, msk1, -BIG)
            nc.vector.tensor_tensor(pen, sg, pen, op=AL.add)
            m2 = gpool.tile([128, GT, 1], F32, name="m2")
            nc.vector.tensor_reduce(m2, pen, axis=mybir.AxisListType.X, op=AL.max)
            dm = gpool.tile([128, GT, 1], F32, name="dm")
            nc.vector.tensor_tensor(dm, m1, m2, op=AL.subtract)
            w1 = gpool.tile([128, GT, 1], F32, name="w1")
            nc.scalar.activation(w1, dm, Sigm, bias=0.0, scale=1.0)
            msk2 = gpool.tile([128, GT, E], F32, name="msk2")
            nc.vector.tensor_tensor(msk2, pen, m2.to_broadcast((128, GT, E)),
                                    op=AL.is_equal)
            t1 = gpool.tile([128, GT, E], F32, name="t1")
            nc.vector.tensor_tensor(t1, msk1, w1.to_broadcast((128, GT, E)),
                                    op=AL.mult)
            w2 = gpool.tile([128, GT, 1], F32, name="w2")
            nc.vector.tensor_scalar(w2, w1, -1.0, 1.0, op0=AL.mult, op1=AL.add)
            t2 = gpool.tile([128, GT, E], F32, name="t2")
            nc.vector.tensor_tensor(t2, msk2, w2.to_broadcast((128, GT, E)),
                                    op=AL.mult)
            wsm = gpool.tile([128, GT, E], F32, name="wsm")
            nc.vector.tensor_tensor(wsm, t1, t2, op=AL.add)
            nc.sync.dma_start(ws_d.ap(), wsm.rearrange("p a b -> p (a b)"))
    nc.compile()
    return nc


def _build_B(caps):
    """One expert per core over gathered tokens in tier chunks (R, B1, P8)."""
    capR, capB, capP = caps
    CT = capR + capB + capP
    nc = bacc.Bacc("TRN2", target_bir_lowering=False, debug=False,
                   num_devices=N_CORES)
    xg_d = nc.dram_tensor("xg", [2, 128, KD, CT], F8, kind="ExternalInput")
    wr_d = nc.dram_tensor("wrow", [CT], F16, kind="ExternalInput")
    V1_d = nc.dram_tensor("V1s", [MH, 128, KDP, 2, 2, 128], F8, kind="ExternalInput")
    V2_d = nc.dram_tensor("V2s", [MH, 128, KHP, 2, 2, 128], F8, kind="ExternalInput")
    V3_d = nc.dram_tensor("V3s", [MO, 128, KHP, 2, 2, 128], F8, kind="ExternalInput")
    C1_d = nc.dram_tensor("C1s", [128, MH], F32, kind="ExternalInput")
    C2_d = nc.dram_tensor("C2s", [128, MH], F32, kind="ExternalInput")
    C3_d = nc.dram_tensor("C3s", [128, MO], F32, kind="ExternalInput")
    outb_d = nc.dram_tensor("outb", [O, CT], F16, kind="ExternalOutput")

    # B1/P8 chunks first: they only need the x hi plane, so compute starts
    # before the lo plane lands (only the R chunk reads it)
    ch1 = [(capR, capB, "ws"), (capR + capB, capP, "p8"), (0, capR, "bs")]
    ch23 = [(capR, capB, "ws"), (capR + capB, capP, "p8"), (0, capR, "ws")]

    with tile.TileContext(nc) as tc:
        with tc.tile_pool(name="const", bufs=1) as cpool, \
             tc.tile_pool(name="acts", bufs=1) as apool, \
             tc.tile_pool(name="wts", bufs=4) as wpool, \
             tc.tile_pool(name="tmp", bufs=4) as tpool, \
             tc.tile_pool(name="ps", bufs=8, space="PSUM") as ps:
            xg = cpool.tile([128, 2, KD, CT], F8, name="xg")
            nc.sync.dma_start(xg[:, 0], xg_d.ap()[0])
            wbc = cpool.tile([128, CT], F16, name="wbc")
            nc.sync.dma_start(wbc, wr_d.ap()[None, :].to_broadcast((128, CT)))
            c1_sb = cpool.tile([128, MH], F32, name="c1_sb")
            nc.sync.dma_start(c1_sb, C1_d.ap())
            c2_sb = cpool.tile([128, MH], F32, name="c2_sb")
            nc.sync.dma_start(c2_sb, C2_d.ap())
            c3_sb = cpool.tile([128, MO], F32, name="c3_sb")
            nc.sync.dma_start(c3_sb, C3_d.ap())
            nc.sync.dma_start(xg[:, 1], xg_d.ap()[1])

            a1 = apool.tile([128, KH, CT], F8, name="a1")
            a2 = apool.tile([128, KH, CT], F8, name="a2")

            def emit1(mi, nsl, pp):
                nc.scalar.activation(a1[:, mi, nsl], pp, Relu,
                                     bias=c1_sb[:, mi:mi + 1], scale=SA / SW)

            _run_layer(nc, wpool, ps, V1_d.ap(), MH, KDP, ch1,
                       lambda j, hl, nsl: xg[:, hl, 2 * j:2 * j + 2, nsl],
                       emit1, "1")

            def emit2(mi, nsl, pp):
                nc.scalar.activation(a2[:, mi, nsl], pp, Relu,
                                     bias=c2_sb[:, mi:mi + 1], scale=1.0 / SW)

            _run_layer(nc, wpool, ps, V2_d.ap(), MH, KHP, ch23,
                       lambda j, hl, nsl: a1[:, 2 * j:2 * j + 2, nsl],
                       emit2, "2")

            # stage per-mi output rows, one bulk DMA per mi
            sls3 = _slices(ch23)
            mov3 = lambda j, hl, nsl: a2[:, 2 * j:2 * j + 2, nsl]
            for mi in range(MO):
                wt = wpool.tile([128, KHP, 2, 2, 128], F8, tag="wt", name="wt3")
                nc.sync.dma_start(wt, V3_d.ap()[mi])
                ob = tpool.tile([128, CT], F16, tag="ob", name="ob")

                def emit3(_mi, nsl, pp, ob=ob):
                    sg = ob[:, nsl]
                    nc.scalar.activation(sg, pp, Sigm,
                                         bias=c3_sb[:, mi:mi + 1],
                                         scale=1.0 / (SA * SW))
                    nc.vector.tensor_tensor(sg, sg, wbc[:, nsl], op=AL.mult)

                for (nsl, mode) in sls3:
                    _mm_group(nc, ps, wt, KHP, mode, mov3, nsl, emit3, mi, "3")
                nc.sync.dma_start(outb_d.ap()[mi * 128:(mi + 1) * 128, :], ob)
    nc.compile()
    return nc


def _build_C():
    """Shared expert layers 2+3 (with a2 hi/lo split) + final combine."""
    nc = bacc.Bacc("TRN2", target_bir_lowering=False, debug=False,
                   num_devices=N_CORES)
    a1_d = nc.dram_tensor("a1s", [TOK // 256, 128, MH, 256], F8,
                          kind="ExternalInput")
    V2_d = nc.dram_tensor("V2s", [MH, 128, KHP, 2, 2, 128], F8, kind="ExternalInput")
    V3_d = nc.dram_tensor("V3s", [MO, 128, KHP, 2, 2, 128], F8, kind="ExternalInput")
    C2_d = nc.dram_tensor("C2s", [128, MH], F32, kind="ExternalInput")
    C3_d = nc.dram_tensor("C3s", [128, MO], F32, kind="ExternalInput")
    c1t_d = nc.dram_tensor("cont1", [O, TOK], F16, kind="ExternalInput")
    c2t_d = nc.dram_tensor("cont2", [O, TOK], F16, kind="ExternalInput")
    out_d = nc.dram_tensor("out", [O, TOK], F32, kind="ExternalOutput")

    NB = TOK // 256
    with tile.TileContext(nc) as tc:
        with tc.tile_pool(name="const", bufs=1) as cpool, \
             tc.tile_pool(name="acts", bufs=1) as apool, \
             tc.tile_pool(name="wts", bufs=4) as wpool, \
             tc.tile_pool(name="tmp", bufs=6) as tpool, \
             tc.tile_pool(name="ps", bufs=8, space="PSUM") as ps:
            a1 = cpool.tile([128, NB, MH, 256], F8, name="a1")
            a1_ap = a1_d.ap().rearrange("b p m t -> p b m t")
            c2_sb = cpool.tile([128, MH], F32, name="c2_sb")
            c3_sb = cpool.tile([128, MO], F32, name="c3_sb")
            # resident layer-2 weights so layer 2 can run block-outer and
            # layer 3 starts as soon as the first a2 block completes;
            # loads ordered by first use
            wt2 = cpool.tile([128, MH, KHP, 2, 2, 128], F8, name="wt2")
            wt2_ap = V2_d.ap().rearrange("a p b c d e -> p a b c d e")
            nc.sync.dma_start(wt2[:, 0:2], wt2_ap[:, 0:2])
            nc.sync.dma_start(a1[:, 0:2], a1_ap[:, 0:2])
            nc.sync.dma_start(c2_sb, C2_d.ap())
            nc.sync.dma_start(c3_sb, C3_d.ap())
            nc.sync.dma_start(wt2[:, 2:5], wt2_ap[:, 2:5])
            nc.sync.dma_start(a1[:, 2:4], a1_ap[:, 2:4])
            nc.sync.dma_start(wt2[:, 5:8], wt2_ap[:, 5:8])
            nc.sync.dma_start(wt2[:, 8:12], wt2_ap[:, 8:12])
            nc.sync.dma_start(wt2[:, 12:16], wt2_ap[:, 12:16])

            a2hl = apool.tile([128, KH, 2, TOK], F8, name="a2hl")

            def emit2(mi, nsl, pp):
                nn = nsl.stop - nsl.start
                t32 = tpool.tile([128, 256], F32, tag="t32", name="t32")[:, :nn]
                nc.scalar.activation(t32, pp, Relu,
                                     bias=c2_sb[:, mi:mi + 1], scale=1.0 / SW)
                nc.vector.tensor_copy(a2hl[:, mi, 0, nsl], t32)
                nc.vector.tensor_tensor(a2hl[:, mi, 1, nsl], t32,
                                        a2hl[:, mi, 0, nsl], op=AL.subtract)

            mov2 = lambda j, hl, nsl: a1[:, nsl.start // 256, 2 * j:2 * j + 2, :]
            for b in range(NB):
                csl = slice(b * 256, (b + 1) * 256)
                for mi in range(MH):
                    _mm_group(nc, ps, wt2[:, mi], KHP, "ws", mov2, csl,
                              emit2, mi, "2")

            mov3 = lambda j, hl, nsl: a2hl[:, 2 * j:2 * j + 2, hl, nsl]
            for mi in range(MO):
                wt = wpool.tile([128, KHP, 2, 2, 128], F8, tag="wt", name="wt3")
                nc.sync.dma_start(wt, V3_d.ap()[mi])
                ct1 = tpool.tile([128, TOK], F16, tag="ct1", name="ct1")
                nc.sync.dma_start(ct1, c1t_d.ap()[mi * 128:(mi + 1) * 128])
                ct2 = tpool.tile([128, TOK], F16, tag="ct2", name="ct2")
                nc.sync.dma_start(ct2, c2t_d.ap()[mi * 128:(mi + 1) * 128])
                ob = tpool.tile([128, TOK], F32, tag="ob", name="ob")

                def emit3(_mi, nsl, pp, ct1=ct1, ct2=ct2, ob=ob, mi=mi):
                    sg = tpool.tile([128, 256], F16, tag="sg", name="sg")
                    nc.scalar.activation(sg, pp, Sigm,
                                         bias=c3_sb[:, mi:mi + 1],
                                         scale=1.0 / (SA * SW))
                    s1 = tpool.tile([128, 256], F16, tag="s1", name="s1")
                    nc.vector.tensor_tensor(s1, sg, ct1[:, nsl], op=AL.add)
                    nc.vector.tensor_tensor(ob[:, nsl], s1, ct2[:, nsl], op=AL.add)

                for b in range(NB):
                    csl = slice(b * 256, (b + 1) * 256)
                    _mm_group(nc, ps, wt, KHP, "bs", mov3, csl, emit3, mi, "3")
                nc.sync.dma_start(out_d.ap()[mi * 128:(mi + 1) * 128, :], ob)
    nc.compile()
    return nc


# ------------------------------------------------------------------ host glue
def _r8(n):
    return max(64, (int(n) + 7) // 8 * 8)


def _route(wsum):
    """Per-expert tier column lists from device gate weights."""
    n = wsum.shape[0]
    e1 = np.argmax(wsum, axis=1)
    w1 = wsum[np.arange(n), e1]
    ws2 = wsum.copy()
    ws2[np.arange(n), e1] = 0.0
    e2 = np.argmax(ws2, axis=1)
    w2 = ws2[np.arange(n), e2]
    tiers = []  # per expert: (tokens, weights, is_first) ordered R,B,P
    nR, nRB, nT = [], [], []
    for e in range(E):
        f = e1 == e
        s = e2 == e
        tR = np.nonzero(f & (w1 > THR1))[0]
        tBf = np.nonzero(f & (w1 <= THR1))[0]
        tBs = np.nonzero(s & (w2 > THR2))[0]
        tP = np.nonzero(s & (w2 <= THR2))[0]
        toks = np.concatenate([tR, tBf, tBs, tP])
        wv = np.concatenate([w1[tR], w1[tBf], w2[tBs], w2[tP]])
        isf = np.concatenate([np.ones(len(tR) + len(tBf), bool),
                              np.zeros(len(tBs) + len(tP), bool)])
        tiers.append((toks, wv, isf))
        nR.append(len(tR))
        nRB.append(len(tR) + len(tBf) + len(tBs))
        nT.append(len(toks))
    # chunk-prefix capacities: tokens placed consecutively may only be
    # upgraded to a higher-precision mode, never downgraded
    capR = _r8(max(nR))
    capRB = max(capR, _r8(max(nRB)))
    CT = max(capRB, _r8(max(nT)))
    return tiers, (capR, capRB - capR, CT - capRB)


_CACHED = {}


def kernel(**inputs) -> np.ndarray:
    inp = {k: np.asarray(v) for k, v in inputs.items()}
    folded = _fold_params(inp)
    x = inp['x'].astype(np.float32)
    WgT = np.ascontiguousarray(inp['Wg'].T.astype(np.float32))
    sh = folded[E]

    # ---- launch A: gate + x split + shared L1 ----
    if "A" not in _CACHED:
        _CACHED["A"] = _build_A()
    ncA = _CACHED["A"]
    mapsA = []
    for c in range(N_CORES):
        xT = np.ascontiguousarray(x[c * TOK:(c + 1) * TOK].T)
        mapsA.append(dict(xT=xT, WgT=WgT, V1s=sh["V1"], C1s=sh["c1"]))
    resA = run_bass_kernel_spmd(ncA, mapsA, core_ids=list(range(N_CORES)))
    wsum = np.concatenate(
        [r["wsum"].reshape(128, GT, E).transpose(1, 0, 2).reshape(TOK, E)
         for r in resA.results], axis=0)
    xcat = np.concatenate([r["xhl"] for r in resA.results], axis=-1)
    a1s = [r["a1s"] for r in resA.results]

    # ---- host dispatch ----
    tiers, caps = _route(wsum)
    capR, capB, capP = caps
    CT = capR + capB + capP

    if _CACHED.get("B_caps") != caps:
        _CACHED["B"] = _build_B(caps)
        _CACHED["B_caps"] = caps
    ncB = _CACHED["B"]
    mapsB = []
    colmaps = []
    for e in range(E):
        toks, wv, isf = tiers[e]
        m = len(toks)
        xg = np.zeros((2, 128, KD, CT), E4)
        xg[0, :, :, :m] = xcat[:, :, 0, toks]
        xg[1, :, :, :m] = xcat[:, :, 1, toks]
        wrow = np.zeros((CT,), np.float16)
        wrow[:m] = wv.astype(np.float16)
        colmaps.append((np.arange(m), toks, isf))
        fe = folded[e]
        mapsB.append(dict(
            xg=xg, wrow=wrow, V1s=fe["V1"], V2s=fe["V2"], V3s=fe["V3"],
            C1s=fe["c1"], C2s=fe["c2"], C3s=fe["c3"]))
    resB = run_bass_kernel_spmd(ncB, mapsB, core_ids=list(range(N_CORES)))

    # ---- host combine alignment (column scatter, channel-major) ----
    cont1 = np.zeros((O, N_TOKENS), np.float16)
    cont2 = np.zeros((O, N_TOKENS), np.float16)
    for e in range(E):
        cols, toks, isf = colmaps[e]
        ob = resB.results[e]["outb"]
        cont1[:, toks[isf]] = ob[:, cols[isf]]
        cont2[:, toks[~isf]] = ob[:, cols[~isf]]

    # ---- launch C: shared L2+L3 + combine ----
    if "C" not in _CACHED:
        _CACHED["C"] = _build_C()
    ncC = _CACHED["C"]
    mapsC = []
    for c in range(N_CORES):
        sl = slice(c * TOK, (c + 1) * TOK)
        mapsC.append(dict(a1s=a1s[c], V2s=sh["V2"], V3s=sh["V3"],
                          C2s=sh["c2"], C3s=sh["c3"],
                          cont1=np.ascontiguousarray(cont1[:, sl]),
                          cont2=np.ascontiguousarray(cont2[:, sl])))
    resC = run_bass_kernel_spmd(ncC, mapsC, core_ids=list(range(N_CORES)))
    out = np.concatenate([np.ascontiguousarray(r["out"].T)
                          for r in resC.results], axis=0)

    _CACHED["timing"] = [(ncA, mapsA), (ncB, mapsB), (ncC, mapsC)]
    return out.astype(np.float32)


# revision 27
# speedup vs baseline: 1.1607x; 1.0364x over previous
"""DeepseekMoE Trainium2 kernel — routed 3-launch pipeline on 8 NeuronCores.

All FFN matmuls run as fp8(e4m3) DoubleRow tensor ops (0.5 cycles/row, 256-wide
contraction per instruction) with per-operand hi/lo residual splits choosing a
precision tier per (token, expert-slot):
  R  (320 cyc/tok): x hi/lo, W hi/lo, a1/a2 single fp8  — high combine weight
  B1 (256 cyc/tok): like R but x single fp8             — mid weight
  P8 (128 cyc/tok): everything single fp8               — low weight (w2<=0.35)
The shared expert runs scheme F (R plus an a2 hi/lo split).  Weight hi/lo
splits are host-side parameter preprocessing; the only data-dependent splits
(x, shared a2) are computed on device.

Launch A (data-parallel): fp32 gate (top-2 via sigmoid identity
  w1 = sigmoid(s1 - s2)), device x hi/lo split, shared-expert layer 1.
Launch B (expert-parallel, one expert per core): 3-layer FFN over
  host-gathered tokens in three tier chunks; outputs weighted fp16.
Launch C (data-parallel): shared layers 2+3 and final combine
  out = shared + cont1 + cont2.
Host code between launches only moves data (gather/scatter/layout); all
per-token arithmetic is on device.
"""
import numpy as np
import ml_dtypes
import concourse.mybir as mybir
import concourse.tile as tile
from concourse import bacc
from concourse.bass_utils import run_bass_kernel_spmd

F32 = mybir.dt.float32
F16 = mybir.dt.float16
F8 = mybir.dt.float8e4
E4 = ml_dtypes.float8_e4m3
DR = mybir.MatmulPerfMode.DoubleRow
AL = mybir.AluOpType
Relu = mybir.ActivationFunctionType.Relu
Sigm = mybir.ActivationFunctionType.Sigmoid

N_TOKENS, D, H, O, E = 8192, 1024, 2048, 1024, 8
N_CORES, TOK = 8, 1024
KD, KH = D // 128, H // 128          # contraction 128-blocks
KDP, KHP = KD // 2, KH // 2          # DoubleRow k-pairs
MH, MO = H // 128, O // 128          # output 128-tiles
GT = TOK // 128                      # gate token tiles per core
EPS = 1e-5
SW, SA = 32.0, 8.0                   # weight / activation fp8 storage scales
BIG = 1e30
THR1, THR2 = 0.6, 0.35               # tier thresholds on combine weight


# ---------------------------------------------------------------- host prep
def _wlayout(V):
    """V [K, M] fp32 -> [MT, 128, KP, 2, 2, 128] e4m3 hi/lo DoubleRow layout.
    k = j*256 + ksub*128 + p ; m = mi*128 + mm ; dim4 = (hi, lo)."""
    Kd, Md = V.shape
    KP, MT = Kd // 256, Md // 128
    s = (V * SW).astype(np.float32)
    hi = s.astype(E4)
    lo = (s - hi.astype(np.float32)).astype(E4)
    out = np.empty((MT, 128, KP, 2, 2, 128), E4)
    out[..., 0, :] = hi.reshape(KP, 2, 128, MT, 128).transpose(3, 2, 0, 1, 4)
    out[..., 1, :] = lo.reshape(KP, 2, 128, MT, 128).transpose(3, 2, 0, 1, 4)
    return np.ascontiguousarray(out)


def _fold_params(inp):
    """Fold eval-mode BN into weights; emit fp8 hi/lo layouts + scaled biases."""
    out = []
    for e in range(E + 1):
        if e < E:
            W1, b1 = inp['W1'][e], inp['b1'][e]
            g1, be1, m1, v1 = inp['g1'][e], inp['be1'][e], inp['m1'][e], inp['v1'][e]
            W2, b2 = inp['W2'][e], inp['b2'][e]
            g2, be2, m2, v2 = inp['g2'][e], inp['be2'][e], inp['m2'][e], inp['v2'][e]
            W3, b3 = inp['W3'][e], inp['b3'][e]
        else:
            W1, b1 = inp['sW1'], inp['sb1']
            g1, be1, m1, v1 = inp['sg1'], inp['sbe1'], inp['sm1'], inp['sv1']
            W2, b2 = inp['sW2'], inp['sb2']
            g2, be2, m2, v2 = inp['sg2'], inp['sbe2'], inp['sm2'], inp['sv2']
            W3, b3 = inp['sW3'], inp['sb3']
        s1 = g1 / np.sqrt(v1 + EPS); t1 = be1 - m1 * s1
        s2 = g2 / np.sqrt(v2 + EPS); t2 = be2 - m2 * s2
        V1 = W1.T.astype(np.float32)
        V2 = (s1[:, None] * W2.T).astype(np.float32)
        V3 = (s2[:, None] * W3.T).astype(np.float32)
        c1 = (SA * b1).astype(np.float32)
        c2 = (SA * (b2 + t1 @ W2.T)).astype(np.float32)
        c3 = (b3 + t2 @ W3.T).astype(np.float32)
        out.append(dict(
            V1=_wlayout(V1), V2=_wlayout(V2), V3=_wlayout(V3),
            c1=np.ascontiguousarray(c1.reshape(MH, 128).T),
            c2=np.ascontiguousarray(c2.reshape(MH, 128).T),
            c3=np.ascontiguousarray(c3.reshape(MO, 128).T)))
    return out


# ------------------------------------------------------------ layer builder
def _mm_group(nc, ps, wt, KP, mode, mov, nsl, emit, mi, tag):
    nn = nsl.stop - nsl.start
    pp = ps.tile([128, 512], F32, tag="ps", name=f"pp{tag}")[:, :nn]
    seq = []
    for j in range(KP):
        hi_st = wt[:, j, :, 0, :]
        lo_st = wt[:, j, :, 1, :]
        if mode == "p8":
            seq.append((hi_st, mov(j, 0, nsl)))
        elif mode == "ws":
            mh = mov(j, 0, nsl)
            seq.append((hi_st, mh))
            seq.append((lo_st, mh))
        else:  # both-split
            mh, ml = mov(j, 0, nsl), mov(j, 1, nsl)
            seq += [(hi_st, mh), (hi_st, ml), (lo_st, mh), (lo_st, ml)]
    for i, (st, mv) in enumerate(seq):
        nc.tensor.matmul(pp, st, mv, start=(i == 0),
                         stop=(i == len(seq) - 1), perf_mode=DR)
    emit(mi, nsl, pp)


def _slices(chunks):
    out = []
    for (off, size, mode) in chunks:
        s = 0
        while s < size:
            nn = min(256, size - s)
            out.append((slice(off + s, off + s + nn), mode))
            s += nn
    return out


def _run_layer(nc, wpool, ps, wdram, MT, KP, chunks, mov, emit, tag,
               post_wt=None):
    """One FFN layer over token chunks (mi-outer, streaming weights).
    chunks: [(off, size, mode)]; mov(j, hl, nsl) -> [128, 2, nn] AP."""
    sls = _slices(chunks)
    for mi in range(MT):
        wt = wpool.tile([128, KP, 2, 2, 128], F8, tag="wt", name=f"wt{tag}")
        nc.sync.dma_start(wt, wdram[mi])
        if post_wt is not None:
            post_wt(mi)
        for (nsl, mode) in sls:
            _mm_group(nc, ps, wt, KP, mode, mov, nsl, emit, mi, tag)


# ------------------------------------------------------------ kernel builders
def _build_A():
    """Gate (fp32) + x hi/lo split + shared-expert layer 1."""
    nc = bacc.Bacc("TRN2", target_bir_lowering=False, debug=False,
                   num_devices=N_CORES)
    xT_d = nc.dram_tensor("xT", [D, TOK], F32, kind="ExternalInput")
    wg_d = nc.dram_tensor("WgT", [D, E], F32, kind="ExternalInput")
    V1_d = nc.dram_tensor("V1s", [MH, 128, KDP, 2, 2, 128], F8, kind="ExternalInput")
    C1_d = nc.dram_tensor("C1s", [128, MH], F32, kind="ExternalInput")
    ws_d = nc.dram_tensor("wsum", [128, GT * E], F32, kind="ExternalOutput")
    xhl_d = nc.dram_tensor("xhl", [128, KD, 2, TOK], F8, kind="ExternalOutput")
    a1s_d = nc.dram_tensor("a1s", [TOK // 256, 128, MH, 256], F8,
                           kind="ExternalOutput")

    NB = TOK // 256
    with tile.TileContext(nc) as tc:
        with tc.tile_pool(name="const", bufs=1) as cpool, \
             tc.tile_pool(name="gate", bufs=1) as gpool, \
             tc.tile_pool(name="ps", bufs=8, space="PSUM") as ps:
            wg = cpool.tile([128, KD, E], F32, name="wg")
            nc.sync.dma_start(wg, wg_d.ap().rearrange("(k p) e -> p k e", p=128))
            c1_sb = cpool.tile([128, MH], F32, name="c1_sb")
            nc.sync.dma_start(c1_sb, C1_d.ap())
            # criticality-ordered loads: x block 0, first weights, then the
            # rest interleaved so layer 1 streams without stalls
            x32 = cpool.tile([128, KD, TOK], F32, name="x32")
            wt1 = cpool.tile([128, MH, KDP, 2, 2, 128], F8, name="wt1")
            xT_ap = xT_d.ap().rearrange("(k p) t -> p k t", p=128)
            wt_ap = V1_d.ap().rearrange("a p b c d e -> p a b c d e")

            def ldx(b):
                csl = slice(b * 256, (b + 1) * 256)
                nc.sync.dma_start(x32[:, :, csl], xT_ap[:, :, csl])

            def ldw(m0, m1_):
                nc.sync.dma_start(wt1[:, m0:m1_], wt_ap[:, m0:m1_])

            ldx(0); ldw(0, 4); ldw(4, 8); ldw(8, 12); ldx(1); ldw(12, 16)
            ldx(2); ldx(3)

            # x hi/lo split (column-blocked: Pool does hi, DVE does lo)
            xhl = cpool.tile([128, KD, 2, TOK], F8, name="xhl")
            for b in range(NB):
                csl = slice(b * 256, (b + 1) * 256)
                for kb in range(KD):
                    nc.gpsimd.tensor_copy(xhl[:, kb, 0, csl], x32[:, kb, csl])
                    nc.vector.tensor_tensor(xhl[:, kb, 1, csl], x32[:, kb, csl],
                                            xhl[:, kb, 0, csl], op=AL.subtract)
            nc.sync.dma_start(xhl_d.ap(), xhl)

            # shared expert layer 1 (both-split), block-outer so output
            # blocks stream out as they complete (block-major staging keeps
            # the outbound DMA descriptors 4 KiB-contiguous)
            a1s = cpool.tile([128, NB, MH, 256], F8, name="a1s")

            def emit1(mi, nsl, pp):
                nc.scalar.activation(a1s[:, nsl.start // 256, mi, :], pp, Relu,
                                     bias=c1_sb[:, mi:mi + 1], scale=SA / SW)

            def emit_gate():
                _emit_gate(nc, gpool, ps, x32, wg, ws_d)

            mov1 = lambda j, hl, nsl: xhl[:, 2 * j:2 * j + 2, hl, nsl]
            for b in range(NB):
                csl = slice(b * 256, (b + 1) * 256)
                for mi in range(MH):
                    _mm_group(nc, ps, wt1[:, mi], KDP, "bs", mov1, csl,
                              emit1, mi, "1")
                nc.sync.dma_start(a1s_d.ap()[b], a1s[:, b])
                if b == NB - 2:
                    # gate mid-stream: its vector tail overlaps the last
                    # layer-1 block; nothing on-device consumes wsum
                    emit_gate()
    nc.compile()
    return nc


def _emit_gate(nc, gpool, ps, x32, wg, ws_d):
            sg = gpool.tile([128, GT, E], F32, name="sg")
            for ti in range(GT):
                tsl = slice(ti * 128, (ti + 1) * 128)
                pg = ps.tile([128, 512], F32, tag="ps", name="pg")[:, :E]
                for kb in range(KD):
                    nc.tensor.matmul(pg, x32[:, kb, tsl], wg[:, kb],
                                     start=(kb == 0), stop=(kb == KD - 1))
                nc.vector.tensor_copy(sg[:, ti], pg)
            m1 = gpool.tile([128, GT, 1], F32, name="m1")
            nc.vector.tensor_reduce(m1, sg, axis=mybir.AxisListType.X, op=AL.max)
            msk1 = gpool.tile([128, GT, E], F32, name="msk1")
            nc.vector.tensor_tensor(msk1, sg, m1.to_broadcast((128, GT, E)),
                                    op=AL.is_equal)
            pen = gpool.tile([128, GT, E], F32, name="pen")
            nc.vector.tensor_scalar_mul(pen, msk1, -BIG)
            nc.vector.tensor_tensor(pen, sg, pen, op=AL.add)
            m2 = gpool.tile([128, GT, 1], F32, name="m2")
            nc.vector.tensor_reduce(m2, pen, axis=mybir.AxisListType.X, op=AL.max)
            dm = gpool.tile([128, GT, 1], F32, name="dm")
            nc.vector.tensor_tensor(dm, m1, m2, op=AL.subtract)
            w1 = gpool.tile([128, GT, 1], F32, name="w1")
            nc.scalar.activation(w1, dm, Sigm, bias=0.0, scale=1.0)
            msk2 = gpool.tile([128, GT, E], F32, name="msk2")
            nc.vector.tensor_tensor(msk2, pen, m2.to_broadcast((128, GT, E)),
                                    op=AL.is_equal)
            t1 = gpool.tile([128, GT, E], F32, name="t1")
            nc.vector.tensor_tensor(t1, msk1, w1.to_broadcast((128, GT, E)),
                                    op=AL.mult)
            w2 = gpool.tile([128, GT, 1], F32, name="w2")
            nc.vector.tensor_scalar(w2, w1, -1.0, 1.0, op0=AL.mult, op1=AL.add)
            t2 = gpool.tile([128, GT, E], F32, name="t2")
            nc.vector.tensor_tensor(t2, msk2, w2.to_broadcast((128, GT, E)),
                                    op=AL.mult)
            wsm = gpool.tile([128, GT, E], F32, name="wsm")
            nc.vector.tensor_tensor(wsm, t1, t2, op=AL.add)
            nc.sync.dma_start(ws_d.ap(), wsm.rearrange("p a b -> p (a b)"))


def _build_B(caps):
    """One expert per core over gathered tokens in tier chunks (R, B1, P8)."""
    capR, capB, capP = caps
    CT = capR + capB + capP
    nc = bacc.Bacc("TRN2", target_bir_lowering=False, debug=False,
                   num_devices=N_CORES)
    xg_d = nc.dram_tensor("xg", [2, 128, KD, CT], F8, kind="ExternalInput")
    wr_d = nc.dram_tensor("wrow", [CT], F16, kind="ExternalInput")
    V1_d = nc.dram_tensor("V1s", [MH, 128, KDP, 2, 2, 128], F8, kind="ExternalInput")
    V2_d = nc.dram_tensor("V2s", [MH, 128, KHP, 2, 2, 128], F8, kind="ExternalInput")
    V3_d = nc.dram_tensor("V3s", [MO, 128, KHP, 2, 2, 128], F8, kind="ExternalInput")
    C1_d = nc.dram_tensor("C1s", [128, MH], F32, kind="ExternalInput")
    C2_d = nc.dram_tensor("C2s", [128, MH], F32, kind="ExternalInput")
    C3_d = nc.dram_tensor("C3s", [128, MO], F32, kind="ExternalInput")
    outb_d = nc.dram_tensor("outb", [O, CT], F16, kind="ExternalOutput")

    # B1/P8 chunks first: they only need the x hi plane, so compute starts
    # before the lo plane lands (only the R chunk reads it)
    ch1 = [(capR, capB, "ws"), (capR + capB, capP, "p8"), (0, capR, "bs")]
    ch23 = [(capR, capB, "ws"), (capR + capB, capP, "p8"), (0, capR, "ws")]

    with tile.TileContext(nc) as tc:
        with tc.tile_pool(name="const", bufs=1) as cpool, \
             tc.tile_pool(name="acts", bufs=1) as apool, \
             tc.tile_pool(name="wts", bufs=4) as wpool, \
             tc.tile_pool(name="tmp", bufs=4) as tpool, \
             tc.tile_pool(name="ps", bufs=8, space="PSUM") as ps:
            xg = cpool.tile([128, 2, KD, CT], F8, name="xg")
            nc.sync.dma_start(xg[:, 0], xg_d.ap()[0])
            c1_sb = cpool.tile([128, MH], F32, name="c1_sb")
            nc.sync.dma_start(c1_sb, C1_d.ap())
            wbc = cpool.tile([128, CT], F16, name="wbc")
            c2_sb = cpool.tile([128, MH], F32, name="c2_sb")
            c3_sb = cpool.tile([128, MO], F32, name="c3_sb")

            a1 = apool.tile([128, KH, CT], F8, name="a1")
            a2 = apool.tile([128, KH, CT], F8, name="a2")

            def emit1(mi, nsl, pp):
                nc.scalar.activation(a1[:, mi, nsl], pp, Relu,
                                     bias=c1_sb[:, mi:mi + 1], scale=SA / SW)

            def post_wt1(mi):
                if mi == 0:
                    nc.sync.dma_start(xg[:, 1], xg_d.ap()[1])
                    nc.sync.dma_start(c2_sb, C2_d.ap())
                    nc.sync.dma_start(c3_sb, C3_d.ap())
                    nc.sync.dma_start(wbc,
                                      wr_d.ap()[None, :].to_broadcast((128, CT)))

            _run_layer(nc, wpool, ps, V1_d.ap(), MH, KDP, ch1,
                       lambda j, hl, nsl: xg[:, hl, 2 * j:2 * j + 2, nsl],
                       emit1, "1", post_wt=post_wt1)

            def emit2(mi, nsl, pp):
                nc.scalar.activation(a2[:, mi, nsl], pp, Relu,
                                     bias=c2_sb[:, mi:mi + 1], scale=1.0 / SW)

            _run_layer(nc, wpool, ps, V2_d.ap(), MH, KHP, ch23,
                       lambda j, hl, nsl: a1[:, 2 * j:2 * j + 2, nsl],
                       emit2, "2")

            # stage per-mi output rows, one bulk DMA per mi
            sls3 = _slices(ch23)
            mov3 = lambda j, hl, nsl: a2[:, 2 * j:2 * j + 2, nsl]
            for mi in range(MO):
                wt = wpool.tile([128, KHP, 2, 2, 128], F8, tag="wt", name="wt3")
                nc.sync.dma_start(wt, V3_d.ap()[mi])
                ob = tpool.tile([128, CT], F16, tag="ob", name="ob")

                def emit3(_mi, nsl, pp, ob=ob):
                    sg = ob[:, nsl]
                    nc.scalar.activation(sg, pp, Sigm,
                                         bias=c3_sb[:, mi:mi + 1],
                                         scale=1.0 / (SA * SW))
                    nc.vector.tensor_tensor(sg, sg, wbc[:, nsl], op=AL.mult)

                for (nsl, mode) in sls3:
                    _mm_group(nc, ps, wt, KHP, mode, mov3, nsl, emit3, mi, "3")
                nc.sync.dma_start(outb_d.ap()[mi * 128:(mi + 1) * 128, :], ob)
    nc.compile()
    return nc


def _build_C():
    """Shared expert layers 2+3 (with a2 hi/lo split) + final combine."""
    nc = bacc.Bacc("TRN2", target_bir_lowering=False, debug=False,
                   num_devices=N_CORES)
    a1_d = nc.dram_tensor("a1s", [TOK // 256, 128, MH, 256], F8,
                          kind="ExternalInput")
    V2_d = nc.dram_tensor("V2s", [MH, 128, KHP, 2, 2, 128], F8, kind="ExternalInput")
    V3_d = nc.dram_tensor("V3s", [MO, 128, KHP, 2, 2, 128], F8, kind="ExternalInput")
    C2_d = nc.dram_tensor("C2s", [128, MH], F32, kind="ExternalInput")
    C3_d = nc.dram_tensor("C3s", [128, MO], F32, kind="ExternalInput")
    c1t_d = nc.dram_tensor("cont1", [O, TOK], F16, kind="ExternalInput")
    c2t_d = nc.dram_tensor("cont2", [O, TOK], F16, kind="ExternalInput")
    out_d = nc.dram_tensor("out", [O, TOK], F32, kind="ExternalOutput")

    NB = TOK // 256
    with tile.TileContext(nc) as tc:
        with tc.tile_pool(name="const", bufs=1) as cpool, \
             tc.tile_pool(name="acts", bufs=1) as apool, \
             tc.tile_pool(name="wts", bufs=4) as wpool, \
             tc.tile_pool(name="tmp", bufs=6) as tpool, \
             tc.tile_pool(name="ps", bufs=8, space="PSUM") as ps:
            a1 = cpool.tile([128, NB, MH, 256], F8, name="a1")
            a1_ap = a1_d.ap().rearrange("b p m t -> p b m t")
            c2_sb = cpool.tile([128, MH], F32, name="c2_sb")
            c3_sb = cpool.tile([128, MO], F32, name="c3_sb")
            nc.sync.dma_start(a1[:, 0:2], a1_ap[:, 0:2])
            nc.sync.dma_start(c2_sb, C2_d.ap())
            nc.sync.dma_start(c3_sb, C3_d.ap())
            nc.sync.dma_start(a1[:, 2:4], a1_ap[:, 2:4])

            a2hl = apool.tile([128, KH, 2, TOK], F8, name="a2hl")

            def emit2(mi, nsl, pp):
                nn = nsl.stop - nsl.start
                t32 = tpool.tile([128, 256], F32, tag="t32", name="t32")[:, :nn]
                nc.scalar.activation(t32, pp, Relu,
                                     bias=c2_sb[:, mi:mi + 1], scale=1.0 / SW)
                nc.vector.tensor_copy(a2hl[:, mi, 0, nsl], t32)
                nc.vector.tensor_tensor(a2hl[:, mi, 1, nsl], t32,
                                        a2hl[:, mi, 0, nsl], op=AL.subtract)

            mov2 = lambda j, hl, nsl: a1[:, nsl.start // 256, 2 * j:2 * j + 2, :]
            _run_layer(nc, wpool, ps, V2_d.ap(), MH, KHP,
                       [(0, TOK, "ws")], mov2, emit2, "2")

            mov3 = lambda j, hl, nsl: a2hl[:, 2 * j:2 * j + 2, hl, nsl]
            for mi in range(MO):
                wt = wpool.tile([128, KHP, 2, 2, 128], F8, tag="wt", name="wt3")
                nc.sync.dma_start(wt, V3_d.ap()[mi])
                ct1 = tpool.tile([128, TOK], F16, tag="ct1", name="ct1")
                nc.sync.dma_start(ct1, c1t_d.ap()[mi * 128:(mi + 1) * 128])
                ct2 = tpool.tile([128, TOK], F16, tag="ct2", name="ct2")
                nc.sync.dma_start(ct2, c2t_d.ap()[mi * 128:(mi + 1) * 128])
                ob = tpool.tile([128, TOK], F32, tag="ob", name="ob")

                def emit3(_mi, nsl, pp, ct1=ct1, ct2=ct2, ob=ob, mi=mi):
                    sg = tpool.tile([128, 256], F16, tag="sg", name="sg")
                    nc.scalar.activation(sg, pp, Sigm,
                                         bias=c3_sb[:, mi:mi + 1],
                                         scale=1.0 / (SA * SW))
                    s1 = tpool.tile([128, 256], F16, tag="s1", name="s1")
                    nc.vector.tensor_tensor(s1, sg, ct1[:, nsl], op=AL.add)
                    nc.vector.tensor_tensor(ob[:, nsl], s1, ct2[:, nsl], op=AL.add)

                for b in range(NB):
                    csl = slice(b * 256, (b + 1) * 256)
                    _mm_group(nc, ps, wt, KHP, "bs", mov3, csl, emit3, mi, "3")
                nc.sync.dma_start(out_d.ap()[mi * 128:(mi + 1) * 128, :], ob)
    nc.compile()
    return nc


# ------------------------------------------------------------------ host glue
def _r8(n):
    return max(64, (int(n) + 7) // 8 * 8)


def _route(wsum):
    """Per-expert tier column lists from device gate weights."""
    n = wsum.shape[0]
    e1 = np.argmax(wsum, axis=1)
    w1 = wsum[np.arange(n), e1]
    ws2 = wsum.copy()
    ws2[np.arange(n), e1] = 0.0
    e2 = np.argmax(ws2, axis=1)
    w2 = ws2[np.arange(n), e2]
    tiers = []  # per expert: (tokens, weights, is_first) ordered R,B,P
    nR, nRB, nT = [], [], []
    for e in range(E):
        f = e1 == e
        s = e2 == e
        tR = np.nonzero(f & (w1 > THR1))[0]
        tBf = np.nonzero(f & (w1 <= THR1))[0]
        tBs = np.nonzero(s & (w2 > THR2))[0]
        tP = np.nonzero(s & (w2 <= THR2))[0]
        toks = np.concatenate([tR, tBf, tBs, tP])
        wv = np.concatenate([w1[tR], w1[tBf], w2[tBs], w2[tP]])
        isf = np.concatenate([np.ones(len(tR) + len(tBf), bool),
                              np.zeros(len(tBs) + len(tP), bool)])
        tiers.append((toks, wv, isf))
        nR.append(len(tR))
        nRB.append(len(tR) + len(tBf) + len(tBs))
        nT.append(len(toks))
    # chunk-prefix capacities: tokens placed consecutively may only be
    # upgraded to a higher-precision mode, never downgraded
    capR = _r8(max(nR))
    capRB = max(capR, _r8(max(nRB)))
    CT = max(capRB, _r8(max(nT)))
    return tiers, (capR, capRB - capR, CT - capRB)


_CACHED = {}


def kernel(**inputs) -> np.ndarray:
    inp = {k: np.asarray(v) for k, v in inputs.items()}
    folded = _fold_params(inp)
    x = inp['x'].astype(np.float32)
    WgT = np.ascontiguousarray(inp['Wg'].T.astype(np.float32))
    sh = folded[E]

    # ---- launch A: gate + x split + shared L1 ----
    if "A" not in _CACHED:
        _CACHED["A"] = _build_A()
    ncA = _CACHED["A"]
    mapsA = []
    for c in range(N_CORES):
        xT = np.ascontiguousarray(x[c * TOK:(c + 1) * TOK].T)
        mapsA.append(dict(xT=xT, WgT=WgT, V1s=sh["V1"], C1s=sh["c1"]))
    resA = run_bass_kernel_spmd(ncA, mapsA, core_ids=list(range(N_CORES)))
    wsum = np.concatenate(
        [r["wsum"].reshape(128, GT, E).transpose(1, 0, 2).reshape(TOK, E)
         for r in resA.results], axis=0)
    xcat = np.concatenate([r["xhl"] for r in resA.results], axis=-1)
    a1s = [r["a1s"] for r in resA.results]

    # ---- host dispatch ----
    tiers, caps = _route(wsum)
    capR, capB, capP = caps
    CT = capR + capB + capP

    if _CACHED.get("B_caps") != caps:
        _CACHED["B"] = _build_B(caps)
        _CACHED["B_caps"] = caps
    ncB = _CACHED["B"]
    mapsB = []
    colmaps = []
    for e in range(E):
        toks, wv, isf = tiers[e]
        m = len(toks)
        xg = np.zeros((2, 128, KD, CT), E4)
        xg[0, :, :, :m] = xcat[:, :, 0, toks]
        xg[1, :, :, :m] = xcat[:, :, 1, toks]
        wrow = np.zeros((CT,), np.float16)
        wrow[:m] = wv.astype(np.float16)
        colmaps.append((np.arange(m), toks, isf))
        fe = folded[e]
        mapsB.append(dict(
            xg=xg, wrow=wrow, V1s=fe["V1"], V2s=fe["V2"], V3s=fe["V3"],
            C1s=fe["c1"], C2s=fe["c2"], C3s=fe["c3"]))
    resB = run_bass_kernel_spmd(ncB, mapsB, core_ids=list(range(N_CORES)))

    # ---- host combine alignment (column scatter, channel-major) ----
    cont1 = np.zeros((O, N_TOKENS), np.float16)
    cont2 = np.zeros((O, N_TOKENS), np.float16)
    for e in range(E):
        cols, toks, isf = colmaps[e]
        ob = resB.results[e]["outb"]
        cont1[:, toks[isf]] = ob[:, cols[isf]]
        cont2[:, toks[~isf]] = ob[:, cols[~isf]]

    # ---- launch C: shared L2+L3 + combine ----
    if "C" not in _CACHED:
        _CACHED["C"] = _build_C()
    ncC = _CACHED["C"]
    mapsC = []
    for c in range(N_CORES):
        sl = slice(c * TOK, (c + 1) * TOK)
        mapsC.append(dict(a1s=a1s[c], V2s=sh["V2"], V3s=sh["V3"],
                          C2s=sh["c2"], C3s=sh["c3"],
                          cont1=np.ascontiguousarray(cont1[:, sl]),
                          cont2=np.ascontiguousarray(cont2[:, sl])))
    resC = run_bass_kernel_spmd(ncC, mapsC, core_ids=list(range(N_CORES)))
    out = np.concatenate([np.ascontiguousarray(r["out"].T)
                          for r in resC.results], axis=0)

    _CACHED["timing"] = [(ncA, mapsA), (ncB, mapsB), (ncC, mapsC)]
    return out.astype(np.float32)


# revision 28
# speedup vs baseline: 1.2043x; 1.0376x over previous
"""DeepseekMoE Trainium2 kernel — routed 3-launch pipeline on 8 NeuronCores.

All FFN matmuls run as fp8(e4m3) DoubleRow tensor ops (0.5 cycles/row, 256-wide
contraction per instruction) with per-operand hi/lo residual splits choosing a
precision tier per (token, expert-slot):
  R  (320 cyc/tok): x hi/lo, W hi/lo, a1/a2 single fp8  — high combine weight
  B1 (256 cyc/tok): like R but x single fp8             — mid weight
  P8 (128 cyc/tok): everything single fp8               — low weight (w2<=0.35)
The shared expert runs scheme F (R plus an a2 hi/lo split).  Weight hi/lo
splits are host-side parameter preprocessing; the only data-dependent splits
(x, shared a2) are computed on device.

Launch A (data-parallel): fp32 gate (top-2 via sigmoid identity
  w1 = sigmoid(s1 - s2)), device x hi/lo split, shared-expert layer 1.
Launch B (expert-parallel, one expert per core): 3-layer FFN over
  host-gathered tokens in three tier chunks; outputs weighted fp16.
Launch C (data-parallel): shared layers 2+3 and final combine
  out = shared + cont1 + cont2.
Host code between launches only moves data (gather/scatter/layout); all
per-token arithmetic is on device.
"""
import numpy as np
import ml_dtypes
import concourse.mybir as mybir
import concourse.tile as tile
from concourse import bacc
from concourse.bass_utils import run_bass_kernel_spmd

F32 = mybir.dt.float32
F16 = mybir.dt.float16
F8 = mybir.dt.float8e4
E4 = ml_dtypes.float8_e4m3
DR = mybir.MatmulPerfMode.DoubleRow
AL = mybir.AluOpType
Relu = mybir.ActivationFunctionType.Relu
Sigm = mybir.ActivationFunctionType.Sigmoid

N_TOKENS, D, H, O, E = 8192, 1024, 2048, 1024, 8
N_CORES, TOK = 8, 1024
KD, KH = D // 128, H // 128          # contraction 128-blocks
KDP, KHP = KD // 2, KH // 2          # DoubleRow k-pairs
MH, MO = H // 128, O // 128          # output 128-tiles
GT = TOK // 128                      # gate token tiles per core
EPS = 1e-5
SW, SA = 32.0, 8.0                   # weight / activation fp8 storage scales
BIG = 1e30
THR1, THR2 = 0.6, 0.40               # tier thresholds on combine weight


# ---------------------------------------------------------------- host prep
def _wlayout(V):
    """V [K, M] fp32 -> [MT, 128, KP, 2, 2, 128] e4m3 hi/lo DoubleRow layout.
    k = j*256 + ksub*128 + p ; m = mi*128 + mm ; dim4 = (hi, lo)."""
    Kd, Md = V.shape
    KP, MT = Kd // 256, Md // 128
    s = (V * SW).astype(np.float32)
    hi = s.astype(E4)
    lo = (s - hi.astype(np.float32)).astype(E4)
    out = np.empty((MT, 128, KP, 2, 2, 128), E4)
    out[..., 0, :] = hi.reshape(KP, 2, 128, MT, 128).transpose(3, 2, 0, 1, 4)
    out[..., 1, :] = lo.reshape(KP, 2, 128, MT, 128).transpose(3, 2, 0, 1, 4)
    return np.ascontiguousarray(out)


def _fold_params(inp):
    """Fold eval-mode BN into weights; emit fp8 hi/lo layouts + scaled biases."""
    out = []
    for e in range(E + 1):
        if e < E:
            W1, b1 = inp['W1'][e], inp['b1'][e]
            g1, be1, m1, v1 = inp['g1'][e], inp['be1'][e], inp['m1'][e], inp['v1'][e]
            W2, b2 = inp['W2'][e], inp['b2'][e]
            g2, be2, m2, v2 = inp['g2'][e], inp['be2'][e], inp['m2'][e], inp['v2'][e]
            W3, b3 = inp['W3'][e], inp['b3'][e]
        else:
            W1, b1 = inp['sW1'], inp['sb1']
            g1, be1, m1, v1 = inp['sg1'], inp['sbe1'], inp['sm1'], inp['sv1']
            W2, b2 = inp['sW2'], inp['sb2']
            g2, be2, m2, v2 = inp['sg2'], inp['sbe2'], inp['sm2'], inp['sv2']
            W3, b3 = inp['sW3'], inp['sb3']
        s1 = g1 / np.sqrt(v1 + EPS); t1 = be1 - m1 * s1
        s2 = g2 / np.sqrt(v2 + EPS); t2 = be2 - m2 * s2
        V1 = W1.T.astype(np.float32)
        V2 = (s1[:, None] * W2.T).astype(np.float32)
        V3 = (s2[:, None] * W3.T).astype(np.float32)
        c1 = (SA * b1).astype(np.float32)
        c2 = (SA * (b2 + t1 @ W2.T)).astype(np.float32)
        c3 = (b3 + t2 @ W3.T).astype(np.float32)
        out.append(dict(
            V1=_wlayout(V1), V2=_wlayout(V2), V3=_wlayout(V3),
            c1=np.ascontiguousarray(c1.reshape(MH, 128).T),
            c2=np.ascontiguousarray(c2.reshape(MH, 128).T),
            c3=np.ascontiguousarray(c3.reshape(MO, 128).T)))
    return out


# ------------------------------------------------------------ layer builder
def _mm_group(nc, ps, wt, KP, mode, mov, nsl, emit, mi, tag):
    nn = nsl.stop - nsl.start
    pp = ps.tile([128, 512], F32, tag="ps", name=f"pp{tag}")[:, :nn]
    seq = []
    for j in range(KP):
        hi_st = wt[:, j, :, 0, :]
        lo_st = wt[:, j, :, 1, :]
        if mode == "p8":
            seq.append((hi_st, mov(j, 0, nsl)))
        elif mode == "ws":
            mh = mov(j, 0, nsl)
            seq.append((hi_st, mh))
            seq.append((lo_st, mh))
        else:  # both-split
            mh, ml = mov(j, 0, nsl), mov(j, 1, nsl)
            seq += [(hi_st, mh), (hi_st, ml), (lo_st, mh), (lo_st, ml)]
    for i, (st, mv) in enumerate(seq):
        nc.tensor.matmul(pp, st, mv, start=(i == 0),
                         stop=(i == len(seq) - 1), perf_mode=DR)
    emit(mi, nsl, pp)


def _slices(chunks):
    out = []
    for (off, size, mode) in chunks:
        s = 0
        while s < size:
            nn = min(256, size - s)
            out.append((slice(off + s, off + s + nn), mode))
            s += nn
    return out


def _run_layer(nc, wpool, ps, wdram, MT, KP, chunks, mov, emit, tag,
               post_wt=None):
    """One FFN layer over token chunks (mi-outer, streaming weights).
    chunks: [(off, size, mode)]; mov(j, hl, nsl) -> [128, 2, nn] AP."""
    sls = _slices(chunks)
    for mi in range(MT):
        wt = wpool.tile([128, KP, 2, 2, 128], F8, tag="wt", name=f"wt{tag}")
        nc.sync.dma_start(wt, wdram[mi])
        if post_wt is not None:
            post_wt(mi)
        for (nsl, mode) in sls:
            _mm_group(nc, ps, wt, KP, mode, mov, nsl, emit, mi, tag)


# ------------------------------------------------------------ kernel builders
def _build_A():
    """Gate (fp32) + x hi/lo split + shared-expert layer 1."""
    nc = bacc.Bacc("TRN2", target_bir_lowering=False, debug=False,
                   num_devices=N_CORES)
    xT_d = nc.dram_tensor("xT", [D, TOK], F32, kind="ExternalInput")
    wg_d = nc.dram_tensor("WgT", [D, E], F32, kind="ExternalInput")
    V1_d = nc.dram_tensor("V1s", [MH, 128, KDP, 2, 2, 128], F8, kind="ExternalInput")
    C1_d = nc.dram_tensor("C1s", [128, MH], F32, kind="ExternalInput")
    ws_d = nc.dram_tensor("wsum", [128, GT * E], F32, kind="ExternalOutput")
    xhl_d = nc.dram_tensor("xhl", [128, KD, 2, TOK], F8, kind="ExternalOutput")
    a1s_d = nc.dram_tensor("a1s", [TOK // 256, 128, MH, 256], F8,
                           kind="ExternalOutput")

    NB = TOK // 256
    with tile.TileContext(nc) as tc:
        with tc.tile_pool(name="const", bufs=1) as cpool, \
             tc.tile_pool(name="gate", bufs=1) as gpool, \
             tc.tile_pool(name="ps", bufs=8, space="PSUM") as ps:
            wg = cpool.tile([128, KD, E], F32, name="wg")
            nc.sync.dma_start(wg, wg_d.ap().rearrange("(k p) e -> p k e", p=128))
            c1_sb = cpool.tile([128, MH], F32, name="c1_sb")
            nc.sync.dma_start(c1_sb, C1_d.ap())
            # criticality-ordered loads: x block 0, first weights, then the
            # rest interleaved so layer 1 streams without stalls
            x32 = cpool.tile([128, KD, TOK], F32, name="x32")
            wt1 = cpool.tile([128, MH, KDP, 2, 2, 128], F8, name="wt1")
            xT_ap = xT_d.ap().rearrange("(k p) t -> p k t", p=128)
            wt_ap = V1_d.ap().rearrange("a p b c d e -> p a b c d e")

            def ldx(b):
                csl = slice(b * 256, (b + 1) * 256)
                nc.sync.dma_start(x32[:, :, csl], xT_ap[:, :, csl])

            def ldw(m0, m1_):
                nc.sync.dma_start(wt1[:, m0:m1_], wt_ap[:, m0:m1_])

            ldx(0); ldw(0, 4); ldw(4, 8); ldw(8, 12); ldx(1); ldw(12, 16)
            ldx(2); ldx(3)

            # x hi/lo split (column-blocked: Pool does hi, DVE does lo)
            xhl = cpool.tile([128, KD, 2, TOK], F8, name="xhl")
            for b in range(NB):
                csl = slice(b * 256, (b + 1) * 256)
                for kb in range(KD):
                    nc.gpsimd.tensor_copy(xhl[:, kb, 0, csl], x32[:, kb, csl])
                    nc.vector.tensor_tensor(xhl[:, kb, 1, csl], x32[:, kb, csl],
                                            xhl[:, kb, 0, csl], op=AL.subtract)
            nc.sync.dma_start(xhl_d.ap(), xhl)

            # shared expert layer 1 (both-split), block-outer so output
            # blocks stream out as they complete (block-major staging keeps
            # the outbound DMA descriptors 4 KiB-contiguous)
            a1s = cpool.tile([128, NB, MH, 256], F8, name="a1s")

            def emit1(mi, nsl, pp):
                nc.scalar.activation(a1s[:, nsl.start // 256, mi, :], pp, Relu,
                                     bias=c1_sb[:, mi:mi + 1], scale=SA / SW)

            def emit_gate():
                _emit_gate(nc, gpool, ps, x32, wg, ws_d)

            mov1 = lambda j, hl, nsl: xhl[:, 2 * j:2 * j + 2, hl, nsl]
            for b in range(NB):
                csl = slice(b * 256, (b + 1) * 256)
                for mi in range(MH):
                    _mm_group(nc, ps, wt1[:, mi], KDP, "bs", mov1, csl,
                              emit1, mi, "1")
                nc.sync.dma_start(a1s_d.ap()[b], a1s[:, b])
                if b == NB - 2:
                    # gate mid-stream: its vector tail overlaps the last
                    # layer-1 block; nothing on-device consumes wsum
                    emit_gate()
    nc.compile()
    return nc


def _emit_gate(nc, gpool, ps, x32, wg, ws_d):
            sg = gpool.tile([128, GT, E], F32, name="sg")
            for ti in range(GT):
                tsl = slice(ti * 128, (ti + 1) * 128)
                pg = ps.tile([128, 512], F32, tag="ps", name="pg")[:, :E]
                for kb in range(KD):
                    nc.tensor.matmul(pg, x32[:, kb, tsl], wg[:, kb],
                                     start=(kb == 0), stop=(kb == KD - 1))
                nc.vector.tensor_copy(sg[:, ti], pg)
            m1 = gpool.tile([128, GT, 1], F32, name="m1")
            nc.vector.tensor_reduce(m1, sg, axis=mybir.AxisListType.X, op=AL.max)
            msk1 = gpool.tile([128, GT, E], F32, name="msk1")
            nc.vector.tensor_tensor(msk1, sg, m1.to_broadcast((128, GT, E)),
                                    op=AL.is_equal)
            pen = gpool.tile([128, GT, E], F32, name="pen")
            nc.vector.tensor_scalar_mul(pen, msk1, -BIG)
            nc.vector.tensor_tensor(pen, sg, pen, op=AL.add)
            m2 = gpool.tile([128, GT, 1], F32, name="m2")
            nc.vector.tensor_reduce(m2, pen, axis=mybir.AxisListType.X, op=AL.max)
            dm = gpool.tile([128, GT, 1], F32, name="dm")
            nc.vector.tensor_tensor(dm, m1, m2, op=AL.subtract)
            w1 = gpool.tile([128, GT, 1], F32, name="w1")
            nc.scalar.activation(w1, dm, Sigm, bias=0.0, scale=1.0)
            msk2 = gpool.tile([128, GT, E], F32, name="msk2")
            nc.vector.tensor_tensor(msk2, pen, m2.to_broadcast((128, GT, E)),
                                    op=AL.is_equal)
            t1 = gpool.tile([128, GT, E], F32, name="t1")
            nc.vector.tensor_tensor(t1, msk1, w1.to_broadcast((128, GT, E)),
                                    op=AL.mult)
            w2 = gpool.tile([128, GT, 1], F32, name="w2")
            nc.vector.tensor_scalar(w2, w1, -1.0, 1.0, op0=AL.mult, op1=AL.add)
            t2 = gpool.tile([128, GT, E], F32, name="t2")
            nc.vector.tensor_tensor(t2, msk2, w2.to_broadcast((128, GT, E)),
                                    op=AL.mult)
            wsm = gpool.tile([128, GT, E], F32, name="wsm")
            nc.vector.tensor_tensor(wsm, t1, t2, op=AL.add)
            nc.sync.dma_start(ws_d.ap(), wsm.rearrange("p a b -> p (a b)"))


def _build_B(caps):
    """One expert per core over gathered tokens in tier prefix chunks
    [0,c1)=R, [c1,c2)=B1, [c2,c3)=B1w3 (layer-3 single-fp8 weights),
    [c3,c4)=P8."""
    c1, c2, c3, c4 = caps
    CT = c4
    nc = bacc.Bacc("TRN2", target_bir_lowering=False, debug=False,
                   num_devices=N_CORES)
    xg_d = nc.dram_tensor("xg", [2, 128, KD, CT], F8, kind="ExternalInput")
    wr_d = nc.dram_tensor("wrow", [CT], F16, kind="ExternalInput")
    V1_d = nc.dram_tensor("V1s", [MH, 128, KDP, 2, 2, 128], F8, kind="ExternalInput")
    V2_d = nc.dram_tensor("V2s", [MH, 128, KHP, 2, 2, 128], F8, kind="ExternalInput")
    V3_d = nc.dram_tensor("V3s", [MO, 128, KHP, 2, 2, 128], F8, kind="ExternalInput")
    C1_d = nc.dram_tensor("C1s", [128, MH], F32, kind="ExternalInput")
    C2_d = nc.dram_tensor("C2s", [128, MH], F32, kind="ExternalInput")
    C3_d = nc.dram_tensor("C3s", [128, MO], F32, kind="ExternalInput")
    outb_d = nc.dram_tensor("outb", [O, CT], F16, kind="ExternalOutput")

    # mid/low chunks first: they only need the x hi plane, so compute
    # starts before the lo plane lands (only the R chunk reads it)
    ch1 = [(c1, c2 - c1, "ws"), (c2, c3 - c2, "ws"), (c3, c4 - c3, "p8"),
           (0, c1, "bs")]
    ch2 = [(c1, c2 - c1, "ws"), (c2, c3 - c2, "ws"), (c3, c4 - c3, "p8"),
           (0, c1, "ws")]
    ch3 = [(c1, c2 - c1, "ws"), (c2, c3 - c2, "p8"), (c3, c4 - c3, "p8"),
           (0, c1, "ws")]

    with tile.TileContext(nc) as tc:
        with tc.tile_pool(name="const", bufs=1) as cpool, \
             tc.tile_pool(name="acts", bufs=1) as apool, \
             tc.tile_pool(name="wts", bufs=4) as wpool, \
             tc.tile_pool(name="tmp", bufs=4) as tpool, \
             tc.tile_pool(name="ps", bufs=8, space="PSUM") as ps:
            xg = cpool.tile([128, 2, KD, CT], F8, name="xg")
            nc.sync.dma_start(xg[:, 0], xg_d.ap()[0])
            c1_sb = cpool.tile([128, MH], F32, name="c1_sb")
            nc.sync.dma_start(c1_sb, C1_d.ap())
            wbc = cpool.tile([128, CT], F16, name="wbc")
            c2_sb = cpool.tile([128, MH], F32, name="c2_sb")
            c3_sb = cpool.tile([128, MO], F32, name="c3_sb")

            a1 = apool.tile([128, KH, CT], F8, name="a1")
            a2 = apool.tile([128, KH, CT], F8, name="a2")

            def emit1(mi, nsl, pp):
                nc.scalar.activation(a1[:, mi, nsl], pp, Relu,
                                     bias=c1_sb[:, mi:mi + 1], scale=SA / SW)

            def post_wt1(mi):
                if mi == 0:
                    nc.sync.dma_start(xg[:, 1], xg_d.ap()[1])
                    nc.sync.dma_start(c2_sb, C2_d.ap())
                    nc.sync.dma_start(c3_sb, C3_d.ap())
                    nc.sync.dma_start(wbc,
                                      wr_d.ap()[None, :].to_broadcast((128, CT)))

            _run_layer(nc, wpool, ps, V1_d.ap(), MH, KDP, ch1,
                       lambda j, hl, nsl: xg[:, hl, 2 * j:2 * j + 2, nsl],
                       emit1, "1", post_wt=post_wt1)

            def emit2(mi, nsl, pp):
                nc.scalar.activation(a2[:, mi, nsl], pp, Relu,
                                     bias=c2_sb[:, mi:mi + 1], scale=1.0 / SW)

            _run_layer(nc, wpool, ps, V2_d.ap(), MH, KHP, ch2,
                       lambda j, hl, nsl: a1[:, 2 * j:2 * j + 2, nsl],
                       emit2, "2")

            # stage per-mi output rows, one bulk DMA per mi
            sls3 = _slices(ch3)
            mov3 = lambda j, hl, nsl: a2[:, 2 * j:2 * j + 2, nsl]
            for mi in range(MO):
                wt = wpool.tile([128, KHP, 2, 2, 128], F8, tag="wt", name="wt3")
                nc.sync.dma_start(wt, V3_d.ap()[mi])
                ob = tpool.tile([128, CT], F16, tag="ob", name="ob")

                def emit3(_mi, nsl, pp, ob=ob):
                    sg = ob[:, nsl]
                    nc.scalar.activation(sg, pp, Sigm,
                                         bias=c3_sb[:, mi:mi + 1],
                                         scale=1.0 / (SA * SW))
                    nc.vector.tensor_tensor(sg, sg, wbc[:, nsl], op=AL.mult)

                for (nsl, mode) in sls3:
                    _mm_group(nc, ps, wt, KHP, mode, mov3, nsl, emit3, mi, "3")
                nc.sync.dma_start(outb_d.ap()[mi * 128:(mi + 1) * 128, :], ob)
    nc.compile()
    return nc


def _build_C():
    """Shared expert layers 2+3 (with a2 hi/lo split) + final combine."""
    nc = bacc.Bacc("TRN2", target_bir_lowering=False, debug=False,
                   num_devices=N_CORES)
    a1_d = nc.dram_tensor("a1s", [TOK // 256, 128, MH, 256], F8,
                          kind="ExternalInput")
    V2_d = nc.dram_tensor("V2s", [MH, 128, KHP, 2, 2, 128], F8, kind="ExternalInput")
    V3_d = nc.dram_tensor("V3s", [MO, 128, KHP, 2, 2, 128], F8, kind="ExternalInput")
    C2_d = nc.dram_tensor("C2s", [128, MH], F32, kind="ExternalInput")
    C3_d = nc.dram_tensor("C3s", [128, MO], F32, kind="ExternalInput")
    c1t_d = nc.dram_tensor("cont1", [O, TOK], F16, kind="ExternalInput")
    c2t_d = nc.dram_tensor("cont2", [O, TOK], F16, kind="ExternalInput")
    out_d = nc.dram_tensor("out", [O, TOK], F32, kind="ExternalOutput")

    NB = TOK // 256
    with tile.TileContext(nc) as tc:
        with tc.tile_pool(name="const", bufs=1) as cpool, \
             tc.tile_pool(name="acts", bufs=1) as apool, \
             tc.tile_pool(name="wts", bufs=4) as wpool, \
             tc.tile_pool(name="tmp", bufs=6) as tpool, \
             tc.tile_pool(name="ps", bufs=8, space="PSUM") as ps:
            a1 = cpool.tile([128, NB, MH, 256], F8, name="a1")
            a1_ap = a1_d.ap().rearrange("b p m t -> p b m t")
            c2_sb = cpool.tile([128, MH], F32, name="c2_sb")
            c3_sb = cpool.tile([128, MO], F32, name="c3_sb")
            nc.sync.dma_start(a1[:, 0:2], a1_ap[:, 0:2])
            nc.sync.dma_start(c2_sb, C2_d.ap())
            nc.sync.dma_start(c3_sb, C3_d.ap())
            nc.sync.dma_start(a1[:, 2:4], a1_ap[:, 2:4])

            a2hl = apool.tile([128, KH, 2, TOK], F8, name="a2hl")

            def emit2(mi, nsl, pp):
                nn = nsl.stop - nsl.start
                t32 = tpool.tile([128, 256], F32, tag="t32", name="t32")[:, :nn]
                nc.scalar.activation(t32, pp, Relu,
                                     bias=c2_sb[:, mi:mi + 1], scale=1.0 / SW)
                nc.vector.tensor_copy(a2hl[:, mi, 0, nsl], t32)
                nc.vector.tensor_tensor(a2hl[:, mi, 1, nsl], t32,
                                        a2hl[:, mi, 0, nsl], op=AL.subtract)

            mov2 = lambda j, hl, nsl: a1[:, nsl.start // 256, 2 * j:2 * j + 2, :]
            _run_layer(nc, wpool, ps, V2_d.ap(), MH, KHP,
                       [(0, TOK, "ws")], mov2, emit2, "2")

            mov3 = lambda j, hl, nsl: a2hl[:, 2 * j:2 * j + 2, hl, nsl]
            for mi in range(MO):
                wt = wpool.tile([128, KHP, 2, 2, 128], F8, tag="wt", name="wt3")
                nc.sync.dma_start(wt, V3_d.ap()[mi])
                ct1 = tpool.tile([128, TOK], F16, tag="ct1", name="ct1")
                nc.sync.dma_start(ct1, c1t_d.ap()[mi * 128:(mi + 1) * 128])
                ct2 = tpool.tile([128, TOK], F16, tag="ct2", name="ct2")
                nc.sync.dma_start(ct2, c2t_d.ap()[mi * 128:(mi + 1) * 128])
                ob = tpool.tile([128, TOK], F32, tag="ob", name="ob")

                def emit3(_mi, nsl, pp, ct1=ct1, ct2=ct2, ob=ob, mi=mi):
                    sg = tpool.tile([128, 256], F16, tag="sg", name="sg")
                    nc.scalar.activation(sg, pp, Sigm,
                                         bias=c3_sb[:, mi:mi + 1],
                                         scale=1.0 / (SA * SW))
                    s1 = tpool.tile([128, 256], F16, tag="s1", name="s1")
                    nc.vector.tensor_tensor(s1, sg, ct1[:, nsl], op=AL.add)
                    nc.vector.tensor_tensor(ob[:, nsl], s1, ct2[:, nsl], op=AL.add)

                for b in range(NB):
                    csl = slice(b * 256, (b + 1) * 256)
                    _mm_group(nc, ps, wt, KHP, "bs", mov3, csl, emit3, mi, "3")
                nc.sync.dma_start(out_d.ap()[mi * 128:(mi + 1) * 128, :], ob)
    nc.compile()
    return nc


# ------------------------------------------------------------------ host glue
def _r8(n):
    return max(64, (int(n) + 7) // 8 * 8)


def _route(wsum):
    """Per-expert tier column lists from device gate weights."""
    n = wsum.shape[0]
    e1 = np.argmax(wsum, axis=1)
    w1 = wsum[np.arange(n), e1]
    ws2 = wsum.copy()
    ws2[np.arange(n), e1] = 0.0
    e2 = np.argmax(ws2, axis=1)
    w2 = ws2[np.arange(n), e2]
    tiers = []  # per expert: (tokens, weights, is_first) ordered R,B1,B1w3,P8
    nR, nRB1, nRB, nT = [], [], [], []
    for e in range(E):
        f = e1 == e
        s = e2 == e
        tR = np.nonzero(f & (w1 > THR1))[0]
        tBf = np.nonzero(f & (w1 <= THR1))[0]
        tBs = np.nonzero(s & (w2 > THR2))[0]
        tP = np.nonzero(s & (w2 <= THR2))[0]
        toks = np.concatenate([tR, tBf, tBs, tP])
        wv = np.concatenate([w1[tR], w1[tBf], w2[tBs], w2[tP]])
        isf = np.concatenate([np.ones(len(tR) + len(tBf), bool),
                              np.zeros(len(tBs) + len(tP), bool)])
        tiers.append((toks, wv, isf))
        nR.append(len(tR))
        nRB1.append(len(tR) + len(tBf))
        nRB.append(len(tR) + len(tBf) + len(tBs))
        nT.append(len(toks))
    # chunk-prefix capacities: tokens placed consecutively may only be
    # upgraded to a higher-precision mode, never downgraded
    c1 = _r8(max(nR))
    c2 = max(c1, _r8(max(nRB1)))
    c3 = max(c2, _r8(max(nRB)))
    c4 = max(c3, _r8(max(nT)))
    return tiers, (c1, c2, c3, c4)


_CACHED = {}


def kernel(**inputs) -> np.ndarray:
    inp = {k: np.asarray(v) for k, v in inputs.items()}
    folded = _fold_params(inp)
    x = inp['x'].astype(np.float32)
    WgT = np.ascontiguousarray(inp['Wg'].T.astype(np.float32))
    sh = folded[E]

    # ---- launch A: gate + x split + shared L1 ----
    if "A" not in _CACHED:
        _CACHED["A"] = _build_A()
    ncA = _CACHED["A"]
    mapsA = []
    for c in range(N_CORES):
        xT = np.ascontiguousarray(x[c * TOK:(c + 1) * TOK].T)
        mapsA.append(dict(xT=xT, WgT=WgT, V1s=sh["V1"], C1s=sh["c1"]))
    resA = run_bass_kernel_spmd(ncA, mapsA, core_ids=list(range(N_CORES)))
    wsum = np.concatenate(
        [r["wsum"].reshape(128, GT, E).transpose(1, 0, 2).reshape(TOK, E)
         for r in resA.results], axis=0)
    xcat = np.concatenate([r["xhl"] for r in resA.results], axis=-1)
    a1s = [r["a1s"] for r in resA.results]

    # ---- host dispatch ----
    tiers, caps = _route(wsum)
    CT = caps[3]

    if _CACHED.get("B_caps") != caps:
        _CACHED["B"] = _build_B(caps)
        _CACHED["B_caps"] = caps
    ncB = _CACHED["B"]
    mapsB = []
    colmaps = []
    for e in range(E):
        toks, wv, isf = tiers[e]
        m = len(toks)
        xg = np.zeros((2, 128, KD, CT), E4)
        xg[0, :, :, :m] = xcat[:, :, 0, toks]
        xg[1, :, :, :m] = xcat[:, :, 1, toks]
        wrow = np.zeros((CT,), np.float16)
        wrow[:m] = wv.astype(np.float16)
        colmaps.append((np.arange(m), toks, isf))
        fe = folded[e]
        mapsB.append(dict(
            xg=xg, wrow=wrow, V1s=fe["V1"], V2s=fe["V2"], V3s=fe["V3"],
            C1s=fe["c1"], C2s=fe["c2"], C3s=fe["c3"]))
    resB = run_bass_kernel_spmd(ncB, mapsB, core_ids=list(range(N_CORES)))

    # ---- host combine alignment (column scatter, channel-major) ----
    cont1 = np.zeros((O, N_TOKENS), np.float16)
    cont2 = np.zeros((O, N_TOKENS), np.float16)
    for e in range(E):
        cols, toks, isf = colmaps[e]
        ob = resB.results[e]["outb"]
        cont1[:, toks[isf]] = ob[:, cols[isf]]
        cont2[:, toks[~isf]] = ob[:, cols[~isf]]

    # ---- launch C: shared L2+L3 + combine ----
    if "C" not in _CACHED:
        _CACHED["C"] = _build_C()
    ncC = _CACHED["C"]
    mapsC = []
    for c in range(N_CORES):
        sl = slice(c * TOK, (c + 1) * TOK)
        mapsC.append(dict(a1s=a1s[c], V2s=sh["V2"], V3s=sh["V3"],
                          C2s=sh["c2"], C3s=sh["c3"],
                          cont1=np.ascontiguousarray(cont1[:, sl]),
                          cont2=np.ascontiguousarray(cont2[:, sl])))
    resC = run_bass_kernel_spmd(ncC, mapsC, core_ids=list(range(N_CORES)))
    out = np.concatenate([np.ascontiguousarray(r["out"].T)
                          for r in resC.results], axis=0)

    _CACHED["timing"] = [(ncA, mapsA), (ncB, mapsB), (ncC, mapsC)]
    return out.astype(np.float32)
